# revision 35
# baseline (speedup 1.0000x reference)
"""Trainium2 Bass kernel for nn_Aggregation (involution-style local aggregation).

out[b, g*64+cw, ho, wo] = sum_{i,j in 5x5} xpad[b, g*64+cw, ho+i, wo+j]
                          * weight[b, cw, i*5+j, ho*64+wo]

Data-parallel over batch: 16 samples -> 8 NeuronCores, 2 samples/core.
Per core:
  - DVE computes the 25 shifted elementwise products (batched 5 window
    shifts per tensor_tensor via an overlapping access pattern),
  - TensorE accumulates them into PSUM with identity-stationary matmuls
    (1 cycle/row),
  - ScalarE evacuates PSUM -> SBUF, DMA writes back.
"""

import os
import sys

import numpy as np

sys.path.insert(0, "/opt/trn_rl_repo")

# Problem constants (hardcoded per contract)
B, C, H, W = 16, 512, 64, 64
CW, KK, KS = 64, 25, 5
PAD = 2
NCORES = 8
S = B // NCORES          # samples per core = 2
PADH = H + 2 * PAD       # 68
NBLK = C // 128          # 4 channel blocks of 128 (each = 2 share-groups)
ROWS = 8                 # output rows per chunk
CHUNK = ROWS * W         # 512 positions = 1 PSUM bank of fp32
NCHUNK = H // ROWS       # 8 chunks per sample

_STATE = {}


def _build_nc():
    import concourse.bass as bass
    import concourse.bacc as bacc
    import concourse.tile as tile
    from concourse import mybir

    f32 = mybir.dt.float32
    f32r = mybir.dt.float32r

    nc = bacc.Bacc("TRN2", target_bir_lowering=False, debug=False, num_devices=NCORES)
    x_in = nc.declare_dram_parameter("input", [S, C, H, W], f32, isOutput=False)
    w_in = nc.declare_dram_parameter("weight", [S, CW, KK, H, W], f32, isOutput=False)
    id_in = nc.declare_dram_parameter("ident", [128, 128], f32, isOutput=False)
    out = nc.declare_dram_parameter("out", [S, C, H, W], f32, isOutput=True)

    with tile.TileContext(nc) as tc:
        with (
            tc.tile_pool(name="const", bufs=1) as constp,
            tc.tile_pool(name="xp", bufs=1) as xpp,
            tc.tile_pool(name="wt", bufs=2) as wtp,
            tc.tile_pool(name="tmp", bufs=2) as tmpp,
            tc.tile_pool(name="ost", bufs=2) as ostp,
            tc.tile_pool(name="ps", bufs=4, space="PSUM") as psp,
        ):
            ident = constp.tile([128, 128], f32)
            nc.sync.dma_start(ident[:], id_in[:])

            # Persistent padded-input tiles, one per channel block. Borders
            # are zeroed once; only the interior is rewritten per sample.
            xp = []
            for cb in range(NBLK):
                t = xpp.tile([128, PADH, PADH], f32, tag=f"xp{cb}")
                nc.vector.memset(t[:], 0.0)
                xp.append(t)

            for b in range(S):
                for cb in range(NBLK):
                    nc.sync.dma_start(
                        xp[cb][:, PAD:PAD + H, PAD:PAD + W],
                        x_in[b, cb * 128:(cb + 1) * 128],
                    )
                for k in range(NCHUNK):
                    wt = wtp.tile([128, KK, ROWS, W], f32)
                    # weight rows for this chunk on partitions 0..63, then
                    # duplicated to 64..127 (channel blocks span 2 groups
                    # sharing the same cw range).
                    nc.sync.dma_start(
                        wt[0:64], w_in[b, :, :, k * ROWS:(k + 1) * ROWS, :]
                    )
                    nc.sync.dma_start(wt[64:128], wt[0:64])
                    for cb in range(NBLK):
                        ps = psp.tile([128, ROWS, W], f32)
                        for j in range(KS):
                            t = tmpp.tile([128, KS, ROWS, W], f32)
                            # x window, batched over the 5 vertical shifts i:
                            # dims (i:5 @ PADH, r:ROWS @ PADH, c:W @ 1),
                            # base offset = (k*ROWS)*PADH + j
                            sl = xp[cb][:, k * ROWS:k * ROWS + ROWS, j:j + W]
                            xov = bass.AP(
                                sl.tensor, sl.offset,
                                [list(sl.ap[0]), [PADH, KS], [PADH, ROWS], [1, W]],
                            )
                            # weight idx = i*5+j for i in 0..5:
                            # offset j*ROWS*W, stride 5*ROWS*W over i
                            wsl = wt[:, j]
                            wov = bass.AP(
                                wsl.tensor, wsl.offset,
                                [list(wsl.ap[0]), [KS * ROWS * W, KS], [W, ROWS], [1, W]],
                            )
                            nc.vector.tensor_mul(t[:], xov, wov)
                            for i in range(KS):
                                idx = i * KS + j
                                nc.tensor.matmul(
                                    ps[:],
                                    ident[:].bitcast(f32r),
                                    t[:, i].bitcast(f32r),
                                    start=(j == 0 and i == 0),
                                    stop=(j == KS - 1 and i == KS - 1),
                                )
                        o = ostp.tile([128, ROWS, W], f32)
                        nc.scalar.copy(o[:], ps[:])
                        nc.sync.dma_start(
                            out[b, cb * 128:(cb + 1) * 128, k * ROWS:(k + 1) * ROWS, :],
                            o[:],
                        )
    nc.compile()
    return nc


def _build_nc_bf16():
    """bf16-products variant (measured ~473 us/core on 8 cores).

    - DVE tensor_tensor runs in 2x_1P mode (2 elem/cycle/lane): every operand
      is bf16, innermost stride 1, 4B-aligned. Odd horizontal shifts j break
      4B alignment, so a second copy of the input, stored shifted by one
      element, serves the odd-j windows.
    - The host supplies the input pre-padded (zero borders, 68x68 per image)
      and pre-cast to bf16 as [S, C, 68*68+1]; the kernel streams it in
      per-chunk row-halo tiles (20 padded rows), fully contiguous transfers
      for both shifted copies.
    - Each tensor_tensor batches the 5 vertical taps of one horizontal shift
      via an overlapping access pattern (free size 5*16*64 = 5120).
    - Products are bf16; the 25-tap accumulation stays exact in fp32 PSUM via
      identity-stationary matmuls (1 cycle/row bf16; identity loads hide
      under the matmul stream). ScalarE evacuates PSUM -> SBUF -> DMA out.
    """
    import concourse.bass as bass
    import concourse.bacc as bacc
    import concourse.tile as tile
    from concourse import mybir

    f32 = mybir.dt.float32
    bf16 = mybir.dt.bfloat16
    NPAD = PADH * PADH   # 4624
    R = 16               # output rows per chunk
    NCH = H // R         # 4 chunks per sample
    HB = R // 2          # rows per PSUM half (512 fp32 = one bank)
    HALO = (R + KS - 1) * PADH  # 20 padded rows = 1360 elements

    nc = bacc.Bacc("TRN2", target_bir_lowering=False, debug=False, num_devices=NCORES)
    x_in = nc.declare_dram_parameter("input", [S, C, NPAD + 1], bf16, isOutput=False)
    w_in = nc.declare_dram_parameter("weight", [S, CW, KK, H, W], bf16, isOutput=False)
    id_in = nc.declare_dram_parameter("ident", [128, 128], bf16, isOutput=False)
    out = nc.declare_dram_parameter("out", [S, C, H, W], f32, isOutput=True)

    with tile.TileContext(nc) as tc:
        with (
            tc.tile_pool(name="const", bufs=1) as constp,
            tc.tile_pool(name="xp", bufs=2) as xpp,
            tc.tile_pool(name="wt", bufs=2) as wtp,
            tc.tile_pool(name="tmp", bufs=5) as tmpp,
            tc.tile_pool(name="ost", bufs=3) as ostp,
            tc.tile_pool(name="ps", bufs=4, space="PSUM") as psp,
        ):
            ident = constp.tile([128, 128], bf16)
            nc.scalar.dma_start(ident[:], id_in[:])

            for b in range(S):
                for k in range(NCH):
                    row0 = k * R * PADH
                    # Two parallel HBM reads of the same weight rows replace
                    # the former SBUF->SBUF partition-duplication DMA, which
                    # serialized behind the x transfers on its FIFO queue and
                    # gated the first products of every chunk. x loads split
                    # across the two HWDGE queues likewise. The very first
                    # chunk streams everything in exact consumption order
                    # (cb0's x, then the 5 tap blocks just-in-time, then the
                    # remaining channel blocks' x) so the DVE product stream
                    # starts ~10us in and never stalls.
                    wsrc = w_in[b, :, :, k * R:(k + 1) * R, :]
                    wt = wtp.tile([128, KK, R, W], bf16, name="wt", tag="wt")
                    xpa, xpb = [], []
                    for cb in range(NBLK):
                        csl = slice(cb * 128, (cb + 1) * 128)
                        ta = xpp.tile([128, HALO], bf16, tag=f"xpa{cb}", name=f"xpa{cb}")
                        tb = xpp.tile([128, HALO], bf16, tag=f"xpb{cb}", name=f"xpb{cb}")
                        xpa.append(ta)
                        xpb.append(tb)
                    first = (b == 0 and k == 0)
                    def load_x(cb):
                        csl = slice(cb * 128, (cb + 1) * 128)
                        nc.sync.dma_start(xpa[cb][:], x_in[b, csl, row0:row0 + HALO])
                        nc.scalar.dma_start(
                            xpb[cb][:], x_in[b, csl, row0 + 1:row0 + 1 + HALO]
                        )
                    if first:
                        load_x(0)
                        for j in range(KS):
                            jb = slice(j * KS, (j + 1) * KS)
                            nc.sync.dma_start(wt[0:64, jb], wsrc[:, jb])
                            nc.scalar.dma_start(wt[64:128, jb], wsrc[:, jb])
                        for cb in range(1, NBLK):
                            load_x(cb)
                    else:
                        for cb in range(NBLK):
                            load_x(cb)
                            if cb == 0:
                                nc.sync.dma_start(wt[0:64], wsrc)
                                nc.scalar.dma_start(wt[64:128], wsrc)
                    for cb in range(NBLK):
                        ps = psp.tile([128, R, W], f32, name="ps", tag="ps")
                        for j in range(KS):
                            t = tmpp.tile([128, KS, R, W], bf16, name="t", tag="t")
                            if j % 2 == 0:
                                xt = xpa[cb][:]
                                base = xt.offset + j
                            else:
                                xt = xpb[cb][:]
                                base = xt.offset + (j - 1)
                            xov = bass.AP(
                                xt.tensor, base,
                                [list(xt.ap[0]), [PADH, KS], [PADH, R], [1, W]],
                            )
                            # weight is j-major on the host: taps for this j
                            # are the contiguous block wt[:, j*5:(j+1)*5]
                            wsl = wt[:, j * KS]
                            wov = bass.AP(
                                wsl.tensor, wsl.offset,
                                [list(wsl.ap[0]), [R * W, KS], [W, R], [1, W]],
                            )
                            nc.vector.tensor_mul(t[:], xov, wov)
                            for half in range(2):
                                for i in range(KS):
                                    nc.tensor.matmul(
                                        ps[:, half * HB:(half + 1) * HB],
                                        ident[:],
                                        t[:, i, half * HB:(half + 1) * HB],
                                        start=(j == 0 and i == 0),
                                        stop=(j == KS - 1 and i == KS - 1),
                                    )
                        # evacuate and store per PSUM half: the lo half's
                        # accumulation closes ~1us before the hi half's, so
                        # its copy and store overlap the hi half's tail
                        o = ostp.tile([128, R, W], f32, name="o", tag="o")
                        for h0, h1 in ((0, HB), (HB, R)):
                            nc.scalar.copy(o[:, h0:h1], ps[:, h0:h1])
                            nc.scalar.dma_start(
                                out[b, cb * 128:(cb + 1) * 128,
                                    k * R + h0:k * R + h1, :],
                                o[:, h0:h1],
                            )
    nc.compile()
    return nc


def _build_nc_v3():
    """Incremental variant over _build_nc_bf16.

    DVE is the bottleneck engine (the 25-tap elementwise products are
    105M bf16 elems/core at 2 elem/cycle/lane = 427us floor; nothing else
    on TRN2 can multiply two position-varying tensors: ScalarE has no
    tensor*tensor, GPSIMD's tensor_tensor is locked out of its shared SBUF
    port while DVE runs, PE needs a stationary operand; the ISA's
    3-free-dim AP cap rules out batching more taps/blocks per DVE
    instruction). Remaining wins are around the DVE stream:

    - bf16 output (host upcasts): halves store traffic.
    - The first sample's first 16 rows are processed as two 8-row
      sub-chunks, halving the data the very first product depends on
      (x halo + weight block), so the DVE starts ~6us earlier.
    - The fill-phase chunk runs its taps even-j-first (0,2,4,1,3), so the
      shifted x copy (xpb, odd j) drops out of the first dependency.
    """
    import concourse.bass as bass
    import concourse.bacc as bacc
    import concourse.tile as tile
    from concourse import mybir

    f32 = mybir.dt.float32
    bf16 = mybir.dt.bfloat16
    NPAD = PADH * PADH   # 4624
    R = 16               # output rows per chunk
    NCH = H // R         # 4 chunks per sample
    HB = R // 2          # rows per PSUM half (512 fp32 = one bank)
    HALO = (R + KS - 1) * PADH  # 20 padded rows = 1360 elements

    nc = bacc.Bacc("TRN2", target_bir_lowering=False, debug=False, num_devices=NCORES)
    x_in = nc.declare_dram_parameter("input", [S, C, NPAD + 1], bf16, isOutput=False)
    w_in = nc.declare_dram_parameter("weight", [S, CW, KK, H, W], bf16, isOutput=False)
    id_in = nc.declare_dram_parameter("ident", [128, 128], bf16, isOutput=False)
    out = nc.declare_dram_parameter("out", [S, C, H, W], bf16, isOutput=True)

    CSTR = NPAD + 1      # channel stride in padded input

    with tile.TileContext(nc) as tc:
        with (
            tc.tile_pool(name="const", bufs=1) as constp,
            tc.tile_pool(name="xp", bufs=2) as xpp,
            tc.tile_pool(name="wt", bufs=2) as wtp,
            tc.tile_pool(name="tmp", bufs=5) as tmpp,
            tc.tile_pool(name="ost", bufs=3) as ostp,
            tc.tile_pool(name="ps", bufs=4, space="PSUM") as psp,
        ):
            ident = constp.tile([128, 128], bf16)
            nc.scalar.dma_start(ident[:], id_in[:])

            # (sample, row0, nrows): the first 16-row chunk is split in two
            # so the pipeline fill waits on half the x/w bytes
            chunks = [(0, 0, 8), (0, 8, 8)]
            chunks += [(0, r, R) for r in range(R, H, R)]
            chunks += [(1, r, R) for r in range(0, H, R)]

            for ci, (b, r0, nr) in enumerate(chunks):
                first = ci == 0
                row0 = r0 * PADH
                halo = (nr + KS - 1) * PADH
                jorder = (0, 2, 4, 1, 3) if ci < 2 else (0, 1, 2, 3, 4)

                xpa, xpb = [], []
                for cb in range(NBLK):
                    xpa.append(xpp.tile([128, HALO], bf16, tag=f"xpa{cb}",
                                        name=f"xpa{cb}"))
                    xpb.append(xpp.tile([128, HALO], bf16, tag=f"xpb{cb}",
                                        name=f"xpb{cb}"))
                wt = wtp.tile([128, KK, R, W], bf16, name="wt", tag="wt")

                def load_x(cb):
                    csl = slice(cb * 128, (cb + 1) * 128)
                    nc.sync.dma_start(
                        xpa[cb][:, :halo], x_in[b, csl, row0:row0 + halo]
                    )
                    nc.scalar.dma_start(
                        xpb[cb][:, :halo], x_in[b, csl, row0 + 1:row0 + 1 + halo]
                    )

                def load_xa(cb):
                    csl = slice(cb * 128, (cb + 1) * 128)
                    nc.sync.dma_start(
                        xpa[cb][:, :halo], x_in[b, csl, row0:row0 + halo]
                    )

                def load_xb(cb):
                    csl = slice(cb * 128, (cb + 1) * 128)
                    nc.scalar.dma_start(
                        xpb[cb][:, :halo], x_in[b, csl, row0 + 1:row0 + 1 + halo]
                    )

                def load_w(j):
                    wsrc = w_in[b, :, j * KS:(j + 1) * KS, r0:r0 + nr, :]
                    jb = slice(j * KS, (j + 1) * KS)
                    nc.sync.dma_start(wt[0:64, jb, :nr], wsrc)
                    nc.scalar.dma_start(wt[64:128, jb, :nr], wsrc)

                if first:
                    # exact consumption order; even-j taps run first, so the
                    # fill needs only xpa0 + the even weight blocks upfront
                    load_xa(0)
                    for j in (0, 2, 4):
                        load_w(j)
                    load_xb(0)
                    for j in (1, 3):
                        load_w(j)
                    for cb in range(1, NBLK):
                        load_x(cb)
                else:
                    for cb in range(NBLK):
                        load_x(cb)
                        if cb == 0:
                            for j in range(KS):
                                load_w(j)

                halves = ((0, HB), (HB, R)) if nr == R else ((0, nr),)
                for cb in range(NBLK):
                    ps = psp.tile([128, R, W], f32, name="ps", tag="ps")
                    for jn, j in enumerate(jorder):
                        t = tmpp.tile([128, KS, R, W], bf16, name="t", tag="t")
                        if j % 2 == 0:
                            xt = xpa[cb][:]
                            base = xt.offset + j
                        else:
                            xt = xpb[cb][:]
                            base = xt.offset + (j - 1)
                        xov = bass.AP(
                            xt.tensor, base,
                            [list(xt.ap[0]), [PADH, KS], [PADH, nr], [1, W]],
                        )
                        wsl = wt[:, j * KS]
                        wov = bass.AP(
                            wsl.tensor, wsl.offset,
                            [list(wsl.ap[0]), [R * W, KS], [W, nr], [1, W]],
                        )
                        nc.vector.tensor_mul(t[:, :, :nr], xov, wov)
                        for h0, h1 in halves:
                            for i in range(KS):
                                nc.tensor.matmul(
                                    ps[:, h0:h1],
                                    ident[:],
                                    t[:, i, h0:h1],
                                    start=(jn == 0 and i == 0),
                                    stop=(jn == KS - 1 and i == KS - 1),
                                )
                    # evacuate per PSUM half (lo closes before hi), cast to
                    # bf16; host upcasts the output
                    o = ostp.tile([128, R, W], bf16, name="o", tag="o")
                    q = nc.sync if cb % 2 == 0 else nc.scalar
                    for h0, h1 in halves:
                        nc.scalar.copy(o[:, h0:h1], ps[:, h0:h1])
                        q.dma_start(
                            out[b, cb * 128:(cb + 1) * 128,
                                r0 + h0:r0 + h1, :],
                            o[:, h0:h1],
                        )
    nc.compile()
    return nc


def _build_nc_v5():
    """Channel-block-merged products via per-j shifted x copies (v5).

    The ISA caps engine APs at 3 free dims, which blocks batching the 4
    channel blocks into one DVE instruction as long as x rows live in
    68-wide padded form ((row, col) then needs its own two dims). Fix: the
    host supplies FIVE shifted copies of the padded input, one per
    horizontal tap j, each with contiguous 64-wide rows. (row, col) then
    collapses into one contiguous dim, and one tensor_mul per (chunk, j)
    covers all 4 channel blocks: free dims [cb=4, i=5, rows*cols], the
    weight operand broadcasting over cb with stride 0. 80 product instrs
    instead of 160 -> half the per-instr init/seq overhead and half the
    semaphore traffic on the critical DVE queue. Every copy is 4B-aligned,
    so the odd-j shifted-copy trick disappears too.

    Costs: x HBM traffic rises to 5 64-wide copies with an 8-row chunk
    halo (63MB/core; 124MB total, still well under the DMA budget) and
    chunks shrink to 8 rows (PSUM: 1 bank per channel block, 8 in
    flight).
    """
    import concourse.bass as bass
    import concourse.bacc as bacc
    import concourse.tile as tile
    from concourse import mybir

    f32 = mybir.dt.float32
    bf16 = mybir.dt.bfloat16
    R = 8                # output rows per chunk (PSUM: 512 f32 = 1 bank/cb)
    NCH = H // R         # 8 chunks per sample
    HALO4 = (R + KS - 1) * W   # 768: 12 rows of 64 in a shifted copy
    FRC = R * W          # 512 contiguous (row, col) elems per (cb, i)

    nc = bacc.Bacc("TRN2", target_bir_lowering=False, debug=False, num_devices=NCORES)
    # input: [S, C, j, padded_row, 64] -- five horizontally-shifted copies
    x_in = nc.declare_dram_parameter(
        "input", [S, C, KS, H + 2 * PAD, W], bf16, isOutput=False
    )
    w_in = nc.declare_dram_parameter("weight", [S, CW, KK, H, W], bf16, isOutput=False)
    id_in = nc.declare_dram_parameter("ident", [128, 128], bf16, isOutput=False)
    out = nc.declare_dram_parameter("out", [S, C, H, W], bf16, isOutput=True)

    with tile.TileContext(nc) as tc:
        with (
            tc.tile_pool(name="const", bufs=1) as constp,
            tc.tile_pool(name="xp", bufs=2) as xpp,
            tc.tile_pool(name="wt", bufs=2) as wtp,
            tc.tile_pool(name="tmp", bufs=3) as tmpp,
            tc.tile_pool(name="ts", bufs=2) as tsp,
            tc.tile_pool(name="ost", bufs=4) as ostp,
            tc.tile_pool(name="ps", bufs=8, space="PSUM") as psp,
        ):
            ident = constp.tile([128, 128], bf16)
            nc.scalar.dma_start(ident[:], id_in[:])

            chunks = [(b, k) for b in range(S) for k in range(NCH)]
            for ci, (b, k) in enumerate(chunks):
                first = ci == 0
                last = ci == len(chunks) - 1
                r0 = k * R

                # x5t[p, j, cb, row*col]: one chunk's halo rows of all five
                # shifted copies, rows contiguous per (j, cb)
                x5t = xpp.tile([128, KS, NBLK, HALO4], bf16, tag="x", name="x5t")
                wt = wtp.tile([128, KK, R, W], bf16, name="wt", tag="wt")

                def load_x(cb, j=None):
                    csl = slice(cb * 128, (cb + 1) * 128)
                    q = nc.sync if cb % 2 == 0 else nc.scalar
                    if j is None:
                        q.dma_start(
                            x5t[:, :, cb], x_in[b, csl, :, r0:r0 + R + KS - 1, :]
                        )
                    else:
                        q.dma_start(
                            x5t[:, j, cb], x_in[b, csl, j, r0:r0 + R + KS - 1, :]
                        )

                def load_w(j):
                    wsrc = w_in[b, :, j * KS:(j + 1) * KS, r0:r0 + R, :]
                    jb = slice(j * KS, (j + 1) * KS)
                    nc.sync.dma_start(wt[0:64, jb], wsrc)
                    nc.scalar.dma_start(wt[64:128, jb], wsrc)

                if first:
                    # consumption order: cb0's j0 slice + j0 weights first
                    load_x(0, 0)
                    load_w(0)
                    for j in range(1, KS):
                        load_x(0, j)
                        load_w(j)
                    for cb in range(1, NBLK):
                        load_x(cb)
                else:
                    for cb in range(NBLK):
                        load_x(cb)
                        if cb == 0:
                            for j in range(KS):
                                load_w(j)

                ps = [
                    psp.tile([128, R, W], f32, name="ps", tag="ps")
                    for _ in range(NBLK)
                ]

                def product_merged(j):
                    t = tmpp.tile([128, NBLK, KS, FRC], bf16, name="t", tag="t")
                    xov = bass.AP(
                        x5t.tensor,
                        x5t.offset + j * NBLK * HALO4,
                        [list(x5t.ap[0]), [HALO4, NBLK], [W, KS], [1, FRC]],
                    )
                    wsl = wt[:, j * KS]
                    wov = bass.AP(
                        wsl.tensor, wsl.offset,
                        [list(wsl.ap[0]), [0, NBLK], [FRC, KS], [1, FRC]],
                    )
                    nc.vector.tensor_mul(t[:], xov, wov)
                    return t

                def product_cb(j, cb):
                    t = tsp.tile([128, KS, FRC], bf16, name="tsg", tag="ts")
                    xov = bass.AP(
                        x5t.tensor,
                        x5t.offset + (j * NBLK + cb) * HALO4,
                        [list(x5t.ap[0]), [W, KS], [1, FRC]],
                    )
                    wsl = wt[:, j * KS]
                    wov = bass.AP(
                        wsl.tensor, wsl.offset,
                        [list(wsl.ap[0]), [FRC, KS], [1, FRC]],
                    )
                    nc.vector.tensor_mul(t[:], xov, wov)
                    return t

                def mm(mv, cb, j, i):
                    nc.tensor.matmul(
                        ps[cb][:],
                        ident[:],
                        mv[:, i],
                        start=(j == 0 and i == 0),
                        stop=(j == KS - 1 and i == KS - 1),
                    )

                def store(cb):
                    o = ostp.tile([128, R, W], bf16, name="o", tag="o")
                    nc.scalar.copy(o[:], ps[cb][:])
                    q = nc.sync if cb % 2 == 0 else nc.scalar
                    q.dma_start(
                        out[b, cb * 128:(cb + 1) * 128, r0:r0 + R, :], o[:]
                    )

                for j in range(KS):
                    split = (first and j == 0) or (last and j == KS - 1)
                    if split:
                        for cb in range(NBLK):
                            t = product_cb(j, cb)
                            for i in range(KS):
                                mm(t, cb, j, i)
                            if j == KS - 1:
                                store(cb)
                    else:
                        t = product_merged(j)
                        for cb in range(NBLK):
                            tcb = t[:, cb]
                            for i in range(KS):
                                mm(tcb, cb, j, i)
                            if j == KS - 1:
                                store(cb)
    nc.compile()
    return nc


def _build_nc_v6():
    """Channel-block-merged products from three 66-wide shifted copies.

    Same idea as v5 (collapse (row, col) into one contiguous AP dim so one
    tensor_mul per (chunk, j) covers all 4 channel blocks, weight operand
    broadcast over cb with stride 0), but with the x-traffic blow-up
    fixed: instead of five 64-wide copies, THREE 66-wide copies serve the
    five horizontal taps at even in-row offsets (j0->copy0+0, j1->copy1+0,
    j2->copy2+0, j3->copy1+2, j4->copy2+2), keeping every operand
    4B-aligned for the DVE 2x mode. Rows are 66 wide, so each product
    carries 2 garbage columns per row (3.1% DVE tax) that the PE moving
    APs simply skip; PSUM tiles stay 64-wide and bank-aligned. x HBM
    traffic is 3*66/(2*68) of the old two-copy scheme (~+10MB/core),
    total DMA ~95MB -- comfortably off the critical path, unlike v5.
    40 merged product instrs instead of 160.
    """
    import concourse.bass as bass
    import concourse.bacc as bacc
    import concourse.tile as tile
    from concourse import mybir

    f32 = mybir.dt.float32
    bf16 = mybir.dt.bfloat16
    R = 16               # output rows per chunk
    NCH = H // R         # 4 chunks per sample
    HB = R // 2          # rows per PSUM half
    WC = 66              # copy row width
    FRC = R * WC         # 1056 contiguous (row, col) elems per (cb, i)
    HALOC = (R + KS - 1) * WC  # 1320: 20 rows of 66 per copy
    NCP = 3
    # j -> (copy, even in-row offset)
    JMAP = {0: (0, 0), 1: (1, 0), 2: (2, 0), 3: (1, 2), 4: (2, 2)}

    nc = bacc.Bacc("TRN2", target_bir_lowering=False, debug=False, num_devices=NCORES)
    # input: three 66-wide shifted copies of the zero-padded image
    x_in = nc.declare_dram_parameter(
        "input", [S, C, NCP, H + 2 * PAD, WC], bf16, isOutput=False
    )
    # weight: taps j-major, rows padded to 66 cols (last 2 cols zero)
    w_in = nc.declare_dram_parameter(
        "weight", [S, CW, KK, H, WC], bf16, isOutput=False
    )
    id_in = nc.declare_dram_parameter("ident", [128, 128], bf16, isOutput=False)
    out = nc.declare_dram_parameter("out", [S, C, H, W], bf16, isOutput=True)

    with tile.TileContext(nc) as tc:
        with (
            tc.tile_pool(name="const", bufs=1) as constp,
            tc.tile_pool(name="xp", bufs=2) as xpp,
            tc.tile_pool(name="wt", bufs=1) as wtp,
            tc.tile_pool(name="tmp", bufs=2) as tmpp,
            tc.tile_pool(name="ost", bufs=2) as ostp,
            tc.tile_pool(name="ps", bufs=4, space="PSUM") as psp,
        ):
            ident = constp.tile([128, 128], bf16)
            nc.scalar.dma_start(ident[:], id_in[:])

            chunks = [(b, k) for b in range(S) for k in range(NCH)]
            for ci, (b, k) in enumerate(chunks):
                first = ci == 0
                last = ci == len(chunks) - 1
                r0 = k * R

                # per-copy tiles [p, cb, row*col], +8 elems of slack for the
                # offset-2 products' tail overrun into garbage
                cps = [
                    xpp.tile([128, NBLK * HALOC + 8], bf16, tag=f"cp{c}",
                             name=f"cp{c}")
                    for c in range(NCP)
                ]
                wts = [
                    wtp.tile([128, KS, FRC], bf16, tag=f"wt{j}", name=f"wt{j}")
                    for j in range(KS)
                ]

                def load_cp(cp, cb0=0, ncb=NBLK):
                    q = (nc.sync, nc.scalar, nc.sync)[cp]
                    dst = cps[cp][:, cb0 * HALOC:(cb0 + ncb) * HALOC]
                    dst = bass.AP(
                        dst.tensor, dst.offset,
                        [list(dst.ap[0]), [HALOC, ncb], [1, HALOC]],
                    )
                    src = x_in[b, :, cp, r0:r0 + R + KS - 1, :]
                    sap = bass.AP(
                        src.tensor, src.offset + cb0 * 128 * NCP * PADH * WC,
                        [[NCP * PADH * WC, 128], [128 * NCP * PADH * WC, ncb],
                         [1, HALOC]],
                    )
                    q.dma_start(dst, sap)

                def load_w(j):
                    wsrc = w_in[b, :, j * KS:(j + 1) * KS, r0:r0 + R, :]
                    nc.sync.dma_start(wts[j][0:64], wsrc)
                    nc.scalar.dma_start(wts[j][64:128], wsrc)

                if first:
                    # cb0's copy-0 slice + j0 weights gate the first product
                    load_cp(0, 0, 1)
                    load_w(0)
                    load_cp(0, 1, 3)
                    load_cp(1)
                    load_w(1)
                    load_cp(2)
                    for j in range(2, KS):
                        load_w(j)
                else:
                    for cp in range(NCP):
                        load_cp(cp)
                    for j in range(KS):
                        load_w(j)

                ps = [
                    psp.tile([128, R, W], f32, name="ps", tag="ps")
                    for _ in range(NBLK)
                ]

                for j in range(KS):
                    cp, ofs = JMAP[j]
                    xt = cps[cp]
                    t = tmpp.tile([128, NBLK, KS, FRC], bf16, name="t", tag="t")
                    wsl = wts[j][:]
                    # fill/drain chunks: issue j0/j4 per channel block (into
                    # slices of the same tile) so the first product waits on
                    # 1/4 of the x bytes and the tail drains 10 matmuls, not
                    # 40
                    split = (first and j == 0) or (last and j == KS - 1)
                    if split:
                        for cb in range(NBLK):
                            xov = bass.AP(
                                xt.tensor, xt.offset + cb * HALOC + ofs,
                                [list(xt.ap[0]), [WC, KS], [1, FRC]],
                            )
                            wov = bass.AP(
                                wsl.tensor, wsl.offset,
                                [list(wsl.ap[0]), [FRC, KS], [1, FRC]],
                            )
                            nc.vector.tensor_mul(t[:, cb], xov, wov)
                    else:
                        xov = bass.AP(
                            xt.tensor, xt.offset + ofs,
                            [list(xt.ap[0]), [HALOC, NBLK], [WC, KS], [1, FRC]],
                        )
                        wov = bass.AP(
                            wsl.tensor, wsl.offset,
                            [list(wsl.ap[0]), [0, NBLK], [FRC, KS], [1, FRC]],
                        )
                        nc.vector.tensor_mul(t[:], xov, wov)
                    for cb in range(NBLK):
                        for half in range(2):
                            for i in range(KS):
                                mv = bass.AP(
                                    t.tensor,
                                    t.offset + (cb * KS + i) * FRC
                                    + half * HB * WC,
                                    [list(t.ap[0]), [WC, HB], [1, W]],
                                )
                                nc.tensor.matmul(
                                    ps[cb][:, half * HB:(half + 1) * HB],
                                    ident[:],
                                    mv,
                                    start=(j == 0 and i == 0),
                                    stop=(j == KS - 1 and i == KS - 1),
                                )
                        if j == KS - 1:
                            o = ostp.tile([128, R, W], bf16, name="o", tag="o")
                            nc.scalar.copy(o[:], ps[cb][:])
                            q = nc.sync if cb % 2 == 0 else nc.scalar
                            q.dma_start(
                                out[b, cb * 128:(cb + 1) * 128, r0:r0 + R, :],
                                o[:],
                            )
    nc.compile()
    return nc


def _build_nc_v7():
    """Samples-on-partitions layout (v7).

    Partition p = (sample b = p//64, cw = p%64) instead of a 128-channel
    block. Wins:
    - The 128 weight partition rows are all DISTINCT (two samples' cw
      rows), so the weight partition-duplication double-read disappears:
      w traffic halves to 26MB/core.
    - The share-group dim g (8 groups whose channels reuse the same cw
      weights) moves to the free dims, so ONE tensor_mul per (chunk, tap
      j) covers the whole chunk: free dims [g=8, i=5, row*col] -- exactly
      the ISA's 3-free-dim cap, with the weight operand broadcast over g
      by stride 0 and (row, col) collapsed via host-side per-j shifted
      64-wide x copies (all offsets 0 -> always 4B-aligned, no odd-shift
      copy). 40 product instrs of FD 20480: ~432us, near the 427us DVE
      floor, vs ~453us for the 160-instr per-channel-block structure.
    - Per-j x and w tiles are consumed by that one instr early in the
      chunk, so they single-buffer; their next-chunk reloads self-stagger
      across the chunk (WAR on the DVE product), smoothing DMA demand.

    8-row chunks: per (chunk, g) PSUM tile [128, 8, 64] f32 = exactly one
    bank, 8 groups = all 8 banks; evacuation per g frees its bank long
    before the next chunk's accumulation reaches it. Products stay bf16
    with fp32 PSUM accumulation; output bf16 (host upcasts).
    """
    import concourse.bass as bass
    import concourse.bacc as bacc
    import concourse.tile as tile
    from concourse import mybir

    f32 = mybir.dt.float32
    bf16 = mybir.dt.bfloat16
    G = C // CW          # 8 share groups
    R = 8                # output rows per chunk (1 PSUM bank per group)
    NCH = H // R         # 8 chunks per sample-pair
    HR = R + KS - 1      # 12 halo rows per chunk
    FRC = R * W          # 512 contiguous (row, col) elems
    FH = HR * W          # 768 halo elems per (j, g)

    nc = bacc.Bacc("TRN2", target_bir_lowering=False, debug=False, num_devices=NCORES)
    # input: [p=(b,cw), g, j, padded_row, 64] -- per-j shifted copies
    x_in = nc.declare_dram_parameter(
        "input", [128, G, KS, H + 2 * PAD, W], bf16, isOutput=False
    )
    # weight: [p=(b,cw), j, i, row, col] -- j-major taps, no duplication
    w_in = nc.declare_dram_parameter(
        "weight", [128, KS, KS, H, W], bf16, isOutput=False
    )
    id_in = nc.declare_dram_parameter("ident", [128, 128], bf16, isOutput=False)
    out = nc.declare_dram_parameter("out", [S, C, H, W], bf16, isOutput=True)

    with tile.TileContext(nc) as tc:
        with (
            tc.tile_pool(name="const", bufs=1) as constp,
            tc.tile_pool(name="xp", bufs=1) as xpp,
            tc.tile_pool(name="wt", bufs=1) as wtp,
            tc.tile_pool(name="tmp", bufs=2) as tmpp,
            tc.tile_pool(name="ost", bufs=8) as ostp,
            tc.tile_pool(name="ps", bufs=8, space="PSUM") as psp,
        ):
            ident = constp.tile([128, 128], bf16)
            nc.scalar.dma_start(ident[:], id_in[:])

            # store DMA triggers deferred into the NEXT chunk's body, after
            # its loads are enqueued: stores resolve only at chunk end (all
            # PSUM groups stop at the last tap), and ahead of the next x/w
            # loads on the FIFO HWDGE queues they head-of-line block the
            # pipeline for ~10us per chunk
            pending_stores = []

            def flush_stores():
                for o, g, r0p in pending_stores:
                    q = nc.sync if g < G // 2 else nc.scalar
                    osl = out[0, g * CW, r0p]
                    q.dma_start(
                        bass.AP(
                            osl.tensor, osl.offset,
                            [[C * H * W, S], [H * W, CW], [1, FRC]],
                        ),
                        o[:],
                    )
                pending_stores.clear()

            for ci in range(NCH):
                first = ci == 0
                last = ci == NCH - 1
                r0 = ci * R

                HG = G // 2   # groups per product half
                xts = [
                    xpp.tile([128, G, FH], bf16, tag=f"x{j}", name=f"x{j}")
                    for j in range(KS)
                ]
                wts = [
                    wtp.tile([128, KS, FRC], bf16, tag=f"w{j}", name=f"w{j}")
                    for j in range(KS)
                ]

                def load_x(j, h):
                    # per-half loads so each half-product waits only on its
                    # own DMA, and the two queues stay balanced
                    q = nc.sync if h == 0 else nc.scalar
                    g0 = h * HG
                    q.dma_start(
                        xts[j][:, g0:g0 + HG],
                        x_in[:, g0:g0 + HG, j, r0:r0 + HR, :],
                    )

                def load_w(j):
                    q = nc.scalar if j % 2 == 0 else nc.sync
                    q.dma_start(wts[j][:], w_in[:, j, :, r0:r0 + R, :])

                if first:
                    load_x(0, 0)
                    load_w(0)
                    load_x(0, 1)
                    for j in range(1, KS):
                        load_x(j, 0)
                        load_w(j)
                        load_x(j, 1)
                else:
                    for j in range(KS):
                        load_x(j, 0)
                        load_x(j, 1)
                        load_w(j)
                flush_stores()

                ps = [
                    psp.tile([128, R, W], f32, name="ps", tag="ps")
                    for _ in range(G)
                ]

                def product(j, h):
                    # half-group product: staggers PE consumption, PSUM
                    # completion and evacuation across the chunk instead of
                    # bunching them at the chunk boundary
                    t = tmpp.tile([128, HG, KS, FRC], bf16, name="t",
                                  tag=f"t{h}")
                    xt = xts[j]
                    xov = bass.AP(
                        xt.tensor, xt.offset + h * HG * FH,
                        [list(xt.ap[0]), [FH, HG], [W, KS], [1, FRC]],
                    )
                    wsl = wts[j][:]
                    wov = bass.AP(
                        wsl.tensor, wsl.offset,
                        [list(wsl.ap[0]), [0, HG], [FRC, KS], [1, FRC]],
                    )
                    nc.vector.tensor_mul(t[:], xov, wov)
                    return t

                def store(g):
                    # evacuate (frees the PSUM bank) now; the HBM store
                    # trigger is deferred past the next chunk's loads
                    o = ostp.tile([128, R, W], bf16, name="o", tag="o")
                    nc.scalar.copy(o[:], ps[g][:])
                    pending_stores.append((o, g, r0))

                for j in range(KS):
                    for h in range(2):
                        t = product(j, h)
                        for g4 in range(HG):
                            g = h * HG + g4
                            for i in range(KS):
                                nc.tensor.matmul(
                                    ps[g][:],
                                    ident[:],
                                    t[:, g4, i],
                                    start=(j == 0 and i == 0),
                                    stop=(j == KS - 1 and i == KS - 1),
                                )
                            if j == KS - 1:
                                store(g)
            flush_stores()
    nc.compile()
    return nc


def _variant():
    return os.environ.get("BASS_KERNEL_VARIANT", "v3")


def _get_nc():
    v = _variant()
    if v not in _STATE:
        if v == "v7":
            _STATE[v] = _build_nc_v7()
        elif v == "v6":
            _STATE[v] = _build_nc_v6()
        elif v == "v5":
            _STATE[v] = _build_nc_v5()
        elif v == "v3":
            _STATE[v] = _build_nc_v3()
        elif v == "bf16":
            _STATE[v] = _build_nc_bf16()
        else:
            _STATE[v] = _build_nc()
    return _STATE[v]


def run(input, weight, trace=False):
    """Run on 8 NeuronCores; returns (output, BassKernelResults)."""
    from concourse.bass_utils import run_bass_kernel_spmd

    assert input.shape == (B, C, H, W), input.shape
    assert weight.shape == (B, CW, KK, H * W), weight.shape
    if _variant() == "v7":
        import ml_dtypes

        dt = ml_dtypes.bfloat16
        # Partition = (sample, cw): per core, [128, g, j, row, 64] with
        # per-j shifted 64-wide copies of the zero-padded image.
        G = C // CW
        xpad = np.zeros((B, C, PADH, PADH), dtype=dt)
        xpad[:, :, PAD:PAD + H, PAD:PAD + W] = np.asarray(
            input, dtype=np.float32
        ).astype(dt)
        xv = xpad.reshape(NCORES, S, G, CW, PADH, PADH)
        xv = np.ascontiguousarray(xv.transpose(0, 1, 3, 2, 4, 5))
        xv = xv.reshape(NCORES, 128, G, PADH, PADH)
        inp = np.empty((NCORES, 128, G, 5, PADH, W), dtype=dt)
        for j in range(5):
            inp[:, :, :, j] = xv[..., j:j + W]
    elif _variant() == "v6":
        import ml_dtypes

        dt = ml_dtypes.bfloat16
        # Three 66-wide shifted copies of the zero-padded input with
        # contiguous rows: the five horizontal taps read them at even
        # in-row offsets, keeping the DVE 2x-mode alignment.
        xpad = np.zeros((B, C, PADH, PADH), dtype=dt)
        xpad[:, :, PAD:PAD + H, PAD:PAD + W] = np.asarray(
            input, dtype=np.float32
        ).astype(dt)
        inp = np.empty((B, C, 3, PADH, 66), dtype=dt)
        for c in range(3):
            inp[:, :, c] = xpad[:, :, :, c:c + 66]
    elif _variant() == "v5":
        import ml_dtypes

        dt = ml_dtypes.bfloat16
        # Five horizontally-shifted copies of the zero-padded input, each
        # with contiguous 64-wide rows: x5[b, c, j, r, :] = xpad[b, c, r,
        # j:j+64]. Lets the kernel collapse (row, col) into one AP dim.
        xpad = np.zeros((B, C, PADH, PADH), dtype=dt)
        xpad[:, :, PAD:PAD + H, PAD:PAD + W] = np.asarray(
            input, dtype=np.float32
        ).astype(dt)
        inp = np.empty((B, C, 5, PADH, W), dtype=dt)
        for j in range(5):
            inp[:, :, j] = xpad[:, :, :, j:j + W]
    elif _variant() in ("bf16", "v3"):
        import ml_dtypes

        dt = ml_dtypes.bfloat16
        # Pre-pad on the host: [B, C, 68*68+1] with zero borders, so the
        # kernel's two shifted SBUF copies are fully contiguous DMAs.
        inp = np.zeros((B, C, PADH * PADH + 1), dtype=dt)
        view = inp[:, :, :PADH * PADH].reshape(B, C, PADH, PADH)
        view[:, :, PAD:PAD + H, PAD:PAD + W] = np.asarray(
            input, dtype=np.float32
        ).astype(dt)
    else:
        dt = np.float32
        inp = np.ascontiguousarray(np.asarray(input, dtype=np.float32))
    wgt = np.asarray(weight, dtype=np.float32).astype(dt).reshape(B, CW, KK, H, W)
    if _variant() in ("bf16", "v3", "v5", "v6", "v7"):
        # j-major tap order: slot j*5+i holds original tap i*5+j, so each
        # horizontal shift's 5 vertical taps are contiguous in SBUF
        wgt = wgt.reshape(B, CW, KS, KS, H, W).transpose(0, 1, 3, 2, 4, 5)
        wgt = wgt.reshape(B, CW, KK, H, W)
    if _variant() == "v6":
        # rows padded to 66 cols (zeros) to match the 66-wide x copies
        w66 = np.zeros((B, CW, KK, H, 66), dtype=dt)
        w66[..., :W] = wgt
        wgt = w66
    if _variant() == "v7":
        # [core, p=(b,cw), j, i, row, col]
        wgt = wgt.reshape(NCORES, 128, KS, KS, H, W)
    wgt = np.ascontiguousarray(wgt)
    ident = np.eye(128, dtype=dt)

    nc = _get_nc()
    core_ids = list(range(NCORES))
    if _variant() == "v7":
        in_maps = [
            {"input": inp[c], "weight": wgt[c], "ident": ident}
            for c in core_ids
        ]
    else:
        in_maps = [
            {
                "input": inp[c * S:(c + 1) * S],
                "weight": wgt[c * S:(c + 1) * S],
                "ident": ident,
            }
            for c in core_ids
        ]
    res = run_bass_kernel_spmd(nc, in_maps, core_ids, trace=trace)
    outp = np.concatenate([res.results[c]["out"] for c in core_ids], axis=0)
    outp = np.ascontiguousarray(outp.astype(np.float32))
    return outp, res


def kernel(input, weight):
    trace = bool(int(os.environ.get("BASS_KERNEL_TRACE", "0")))
    outp, _ = run(input, weight, trace=trace)
    return outp



# revision 39
# speedup vs baseline: 1.0427x; 1.0427x over previous
"""Trainium2 Bass kernel for nn_Aggregation (involution-style local aggregation).

out[b, g*64+cw, ho, wo] = sum_{i,j in 5x5} xpad[b, g*64+cw, ho+i, wo+j]
                          * weight[b, cw, i*5+j, ho*64+wo]

Data-parallel over batch: 16 samples -> 8 NeuronCores, 2 samples/core.
Per core:
  - DVE computes the 25 shifted elementwise products (batched 5 window
    shifts per tensor_tensor via an overlapping access pattern),
  - TensorE accumulates them into PSUM with identity-stationary matmuls
    (1 cycle/row),
  - ScalarE evacuates PSUM -> SBUF, DMA writes back.
"""

import os
import sys

import numpy as np

sys.path.insert(0, "/opt/trn_rl_repo")

# Problem constants (hardcoded per contract)
B, C, H, W = 16, 512, 64, 64
CW, KK, KS = 64, 25, 5
PAD = 2
NCORES = 8
S = B // NCORES          # samples per core = 2
PADH = H + 2 * PAD       # 68
NBLK = C // 128          # 4 channel blocks of 128 (each = 2 share-groups)
ROWS = 8                 # output rows per chunk
CHUNK = ROWS * W         # 512 positions = 1 PSUM bank of fp32
NCHUNK = H // ROWS       # 8 chunks per sample

_STATE = {}


def _build_nc():
    import concourse.bass as bass
    import concourse.bacc as bacc
    import concourse.tile as tile
    from concourse import mybir

    f32 = mybir.dt.float32
    f32r = mybir.dt.float32r

    nc = bacc.Bacc("TRN2", target_bir_lowering=False, debug=False, num_devices=NCORES)
    x_in = nc.declare_dram_parameter("input", [S, C, H, W], f32, isOutput=False)
    w_in = nc.declare_dram_parameter("weight", [S, CW, KK, H, W], f32, isOutput=False)
    id_in = nc.declare_dram_parameter("ident", [128, 128], f32, isOutput=False)
    out = nc.declare_dram_parameter("out", [S, C, H, W], f32, isOutput=True)

    with tile.TileContext(nc) as tc:
        with (
            tc.tile_pool(name="const", bufs=1) as constp,
            tc.tile_pool(name="xp", bufs=1) as xpp,
            tc.tile_pool(name="wt", bufs=2) as wtp,
            tc.tile_pool(name="tmp", bufs=2) as tmpp,
            tc.tile_pool(name="ost", bufs=2) as ostp,
            tc.tile_pool(name="ps", bufs=4, space="PSUM") as psp,
        ):
            ident = constp.tile([128, 128], f32)
            nc.sync.dma_start(ident[:], id_in[:])

            # Persistent padded-input tiles, one per channel block. Borders
            # are zeroed once; only the interior is rewritten per sample.
            xp = []
            for cb in range(NBLK):
                t = xpp.tile([128, PADH, PADH], f32, tag=f"xp{cb}")
                nc.vector.memset(t[:], 0.0)
                xp.append(t)

            for b in range(S):
                for cb in range(NBLK):
                    nc.sync.dma_start(
                        xp[cb][:, PAD:PAD + H, PAD:PAD + W],
                        x_in[b, cb * 128:(cb + 1) * 128],
                    )
                for k in range(NCHUNK):
                    wt = wtp.tile([128, KK, ROWS, W], f32)
                    # weight rows for this chunk on partitions 0..63, then
                    # duplicated to 64..127 (channel blocks span 2 groups
                    # sharing the same cw range).
                    nc.sync.dma_start(
                        wt[0:64], w_in[b, :, :, k * ROWS:(k + 1) * ROWS, :]
                    )
                    nc.sync.dma_start(wt[64:128], wt[0:64])
                    for cb in range(NBLK):
                        ps = psp.tile([128, ROWS, W], f32)
                        for j in range(KS):
                            t = tmpp.tile([128, KS, ROWS, W], f32)
                            # x window, batched over the 5 vertical shifts i:
                            # dims (i:5 @ PADH, r:ROWS @ PADH, c:W @ 1),
                            # base offset = (k*ROWS)*PADH + j
                            sl = xp[cb][:, k * ROWS:k * ROWS + ROWS, j:j + W]
                            xov = bass.AP(
                                sl.tensor, sl.offset,
                                [list(sl.ap[0]), [PADH, KS], [PADH, ROWS], [1, W]],
                            )
                            # weight idx = i*5+j for i in 0..5:
                            # offset j*ROWS*W, stride 5*ROWS*W over i
                            wsl = wt[:, j]
                            wov = bass.AP(
                                wsl.tensor, wsl.offset,
                                [list(wsl.ap[0]), [KS * ROWS * W, KS], [W, ROWS], [1, W]],
                            )
                            nc.vector.tensor_mul(t[:], xov, wov)
                            for i in range(KS):
                                idx = i * KS + j
                                nc.tensor.matmul(
                                    ps[:],
                                    ident[:].bitcast(f32r),
                                    t[:, i].bitcast(f32r),
                                    start=(j == 0 and i == 0),
                                    stop=(j == KS - 1 and i == KS - 1),
                                )
                        o = ostp.tile([128, ROWS, W], f32)
                        nc.scalar.copy(o[:], ps[:])
                        nc.sync.dma_start(
                            out[b, cb * 128:(cb + 1) * 128, k * ROWS:(k + 1) * ROWS, :],
                            o[:],
                        )
    nc.compile()
    return nc


def _build_nc_bf16():
    """bf16-products variant (measured ~473 us/core on 8 cores).

    - DVE tensor_tensor runs in 2x_1P mode (2 elem/cycle/lane): every operand
      is bf16, innermost stride 1, 4B-aligned. Odd horizontal shifts j break
      4B alignment, so a second copy of the input, stored shifted by one
      element, serves the odd-j windows.
    - The host supplies the input pre-padded (zero borders, 68x68 per image)
      and pre-cast to bf16 as [S, C, 68*68+1]; the kernel streams it in
      per-chunk row-halo tiles (20 padded rows), fully contiguous transfers
      for both shifted copies.
    - Each tensor_tensor batches the 5 vertical taps of one horizontal shift
      via an overlapping access pattern (free size 5*16*64 = 5120).
    - Products are bf16; the 25-tap accumulation stays exact in fp32 PSUM via
      identity-stationary matmuls (1 cycle/row bf16; identity loads hide
      under the matmul stream). ScalarE evacuates PSUM -> SBUF -> DMA out.
    """
    import concourse.bass as bass
    import concourse.bacc as bacc
    import concourse.tile as tile
    from concourse import mybir

    f32 = mybir.dt.float32
    bf16 = mybir.dt.bfloat16
    NPAD = PADH * PADH   # 4624
    R = 16               # output rows per chunk
    NCH = H // R         # 4 chunks per sample
    HB = R // 2          # rows per PSUM half (512 fp32 = one bank)
    HALO = (R + KS - 1) * PADH  # 20 padded rows = 1360 elements

    nc = bacc.Bacc("TRN2", target_bir_lowering=False, debug=False, num_devices=NCORES)
    x_in = nc.declare_dram_parameter("input", [S, C, NPAD + 1], bf16, isOutput=False)
    w_in = nc.declare_dram_parameter("weight", [S, CW, KK, H, W], bf16, isOutput=False)
    id_in = nc.declare_dram_parameter("ident", [128, 128], bf16, isOutput=False)
    out = nc.declare_dram_parameter("out", [S, C, H, W], f32, isOutput=True)

    with tile.TileContext(nc) as tc:
        with (
            tc.tile_pool(name="const", bufs=1) as constp,
            tc.tile_pool(name="xp", bufs=2) as xpp,
            tc.tile_pool(name="wt", bufs=2) as wtp,
            tc.tile_pool(name="tmp", bufs=5) as tmpp,
            tc.tile_pool(name="ost", bufs=3) as ostp,
            tc.tile_pool(name="ps", bufs=4, space="PSUM") as psp,
        ):
            ident = constp.tile([128, 128], bf16)
            nc.scalar.dma_start(ident[:], id_in[:])

            for b in range(S):
                for k in range(NCH):
                    row0 = k * R * PADH
                    # Two parallel HBM reads of the same weight rows replace
                    # the former SBUF->SBUF partition-duplication DMA, which
                    # serialized behind the x transfers on its FIFO queue and
                    # gated the first products of every chunk. x loads split
                    # across the two HWDGE queues likewise. The very first
                    # chunk streams everything in exact consumption order
                    # (cb0's x, then the 5 tap blocks just-in-time, then the
                    # remaining channel blocks' x) so the DVE product stream
                    # starts ~10us in and never stalls.
                    wsrc = w_in[b, :, :, k * R:(k + 1) * R, :]
                    wt = wtp.tile([128, KK, R, W], bf16, name="wt", tag="wt")
                    xpa, xpb = [], []
                    for cb in range(NBLK):
                        csl = slice(cb * 128, (cb + 1) * 128)
                        ta = xpp.tile([128, HALO], bf16, tag=f"xpa{cb}", name=f"xpa{cb}")
                        tb = xpp.tile([128, HALO], bf16, tag=f"xpb{cb}", name=f"xpb{cb}")
                        xpa.append(ta)
                        xpb.append(tb)
                    first = (b == 0 and k == 0)
                    def load_x(cb):
                        csl = slice(cb * 128, (cb + 1) * 128)
                        nc.sync.dma_start(xpa[cb][:], x_in[b, csl, row0:row0 + HALO])
                        nc.scalar.dma_start(
                            xpb[cb][:], x_in[b, csl, row0 + 1:row0 + 1 + HALO]
                        )
                    if first:
                        load_x(0)
                        for j in range(KS):
                            jb = slice(j * KS, (j + 1) * KS)
                            nc.sync.dma_start(wt[0:64, jb], wsrc[:, jb])
                            nc.scalar.dma_start(wt[64:128, jb], wsrc[:, jb])
                        for cb in range(1, NBLK):
                            load_x(cb)
                    else:
                        for cb in range(NBLK):
                            load_x(cb)
                            if cb == 0:
                                nc.sync.dma_start(wt[0:64], wsrc)
                                nc.scalar.dma_start(wt[64:128], wsrc)
                    for cb in range(NBLK):
                        ps = psp.tile([128, R, W], f32, name="ps", tag="ps")
                        for j in range(KS):
                            t = tmpp.tile([128, KS, R, W], bf16, name="t", tag="t")
                            if j % 2 == 0:
                                xt = xpa[cb][:]
                                base = xt.offset + j
                            else:
                                xt = xpb[cb][:]
                                base = xt.offset + (j - 1)
                            xov = bass.AP(
                                xt.tensor, base,
                                [list(xt.ap[0]), [PADH, KS], [PADH, R], [1, W]],
                            )
                            # weight is j-major on the host: taps for this j
                            # are the contiguous block wt[:, j*5:(j+1)*5]
                            wsl = wt[:, j * KS]
                            wov = bass.AP(
                                wsl.tensor, wsl.offset,
                                [list(wsl.ap[0]), [R * W, KS], [W, R], [1, W]],
                            )
                            nc.vector.tensor_mul(t[:], xov, wov)
                            for half in range(2):
                                for i in range(KS):
                                    nc.tensor.matmul(
                                        ps[:, half * HB:(half + 1) * HB],
                                        ident[:],
                                        t[:, i, half * HB:(half + 1) * HB],
                                        start=(j == 0 and i == 0),
                                        stop=(j == KS - 1 and i == KS - 1),
                                    )
                        # evacuate and store per PSUM half: the lo half's
                        # accumulation closes ~1us before the hi half's, so
                        # its copy and store overlap the hi half's tail
                        o = ostp.tile([128, R, W], f32, name="o", tag="o")
                        for h0, h1 in ((0, HB), (HB, R)):
                            nc.scalar.copy(o[:, h0:h1], ps[:, h0:h1])
                            nc.scalar.dma_start(
                                out[b, cb * 128:(cb + 1) * 128,
                                    k * R + h0:k * R + h1, :],
                                o[:, h0:h1],
                            )
    nc.compile()
    return nc


def _build_nc_v3():
    """Incremental variant over _build_nc_bf16.

    DVE is the bottleneck engine (the 25-tap elementwise products are
    105M bf16 elems/core at 2 elem/cycle/lane = 427us floor; nothing else
    on TRN2 can multiply two position-varying tensors: ScalarE has no
    tensor*tensor, GPSIMD's tensor_tensor is locked out of its shared SBUF
    port while DVE runs, PE needs a stationary operand; the ISA's
    3-free-dim AP cap rules out batching more taps/blocks per DVE
    instruction). Remaining wins are around the DVE stream:

    - bf16 output (host upcasts): halves store traffic.
    - The first sample's first 16 rows are processed as two 8-row
      sub-chunks, halving the data the very first product depends on
      (x halo + weight block), so the DVE starts ~6us earlier.
    - The fill-phase chunk runs its taps even-j-first (0,2,4,1,3), so the
      shifted x copy (xpb, odd j) drops out of the first dependency.
    """
    import concourse.bass as bass
    import concourse.bacc as bacc
    import concourse.tile as tile
    from concourse import mybir

    f32 = mybir.dt.float32
    bf16 = mybir.dt.bfloat16
    NPAD = PADH * PADH   # 4624
    R = 16               # output rows per chunk
    NCH = H // R         # 4 chunks per sample
    HB = R // 2          # rows per PSUM half (512 fp32 = one bank)
    HALO = (R + KS - 1) * PADH  # 20 padded rows = 1360 elements

    nc = bacc.Bacc("TRN2", target_bir_lowering=False, debug=False, num_devices=NCORES)
    x_in = nc.declare_dram_parameter("input", [S, C, NPAD + 1], bf16, isOutput=False)
    w_in = nc.declare_dram_parameter("weight", [S, CW, KK, H, W], bf16, isOutput=False)
    id_in = nc.declare_dram_parameter("ident", [128, 128], bf16, isOutput=False)
    out = nc.declare_dram_parameter("out", [S, C, H, W], bf16, isOutput=True)

    CSTR = NPAD + 1      # channel stride in padded input

    with tile.TileContext(nc) as tc:
        with (
            tc.tile_pool(name="const", bufs=1) as constp,
            tc.tile_pool(name="xp", bufs=2) as xpp,
            tc.tile_pool(name="wt", bufs=2) as wtp,
            tc.tile_pool(name="tmp", bufs=5) as tmpp,
            tc.tile_pool(name="ost", bufs=3) as ostp,
            tc.tile_pool(name="ps", bufs=4, space="PSUM") as psp,
        ):
            ident = constp.tile([128, 128], bf16)
            nc.scalar.dma_start(ident[:], id_in[:])

            # (sample, row0, nrows): the first 16-row chunk is split in two
            # so the pipeline fill waits on half the x/w bytes
            chunks = [(0, 0, 8), (0, 8, 8)]
            chunks += [(0, r, R) for r in range(R, H, R)]
            chunks += [(1, r, R) for r in range(0, H, R)]

            for ci, (b, r0, nr) in enumerate(chunks):
                first = ci == 0
                row0 = r0 * PADH
                halo = (nr + KS - 1) * PADH
                jorder = (0, 2, 4, 1, 3) if ci < 2 else (0, 1, 2, 3, 4)

                xpa, xpb = [], []
                for cb in range(NBLK):
                    xpa.append(xpp.tile([128, HALO], bf16, tag=f"xpa{cb}",
                                        name=f"xpa{cb}"))
                    xpb.append(xpp.tile([128, HALO], bf16, tag=f"xpb{cb}",
                                        name=f"xpb{cb}"))
                wt = wtp.tile([128, KK, R, W], bf16, name="wt", tag="wt")

                def load_x(cb):
                    csl = slice(cb * 128, (cb + 1) * 128)
                    nc.sync.dma_start(
                        xpa[cb][:, :halo], x_in[b, csl, row0:row0 + halo]
                    )
                    nc.scalar.dma_start(
                        xpb[cb][:, :halo], x_in[b, csl, row0 + 1:row0 + 1 + halo]
                    )

                def load_xa(cb):
                    csl = slice(cb * 128, (cb + 1) * 128)
                    nc.sync.dma_start(
                        xpa[cb][:, :halo], x_in[b, csl, row0:row0 + halo]
                    )

                def load_xb(cb):
                    csl = slice(cb * 128, (cb + 1) * 128)
                    nc.scalar.dma_start(
                        xpb[cb][:, :halo], x_in[b, csl, row0 + 1:row0 + 1 + halo]
                    )

                def load_w(j):
                    wsrc = w_in[b, :, j * KS:(j + 1) * KS, r0:r0 + nr, :]
                    jb = slice(j * KS, (j + 1) * KS)
                    nc.sync.dma_start(wt[0:64, jb, :nr], wsrc)
                    nc.scalar.dma_start(wt[64:128, jb, :nr], wsrc)

                if first:
                    # exact consumption order; even-j taps run first, so the
                    # fill needs only xpa0 + the even weight blocks upfront
                    load_xa(0)
                    for j in (0, 2, 4):
                        load_w(j)
                    load_xb(0)
                    for j in (1, 3):
                        load_w(j)
                    for cb in range(1, NBLK):
                        load_x(cb)
                else:
                    for cb in range(NBLK):
                        load_x(cb)
                        if cb == 0:
                            for j in range(KS):
                                load_w(j)

                halves = ((0, HB), (HB, R)) if nr == R else ((0, nr),)
                for cb in range(NBLK):
                    ps = psp.tile([128, R, W], f32, name="ps", tag="ps")
                    for jn, j in enumerate(jorder):
                        t = tmpp.tile([128, KS, R, W], bf16, name="t", tag="t")
                        if j % 2 == 0:
                            xt = xpa[cb][:]
                            base = xt.offset + j
                        else:
                            xt = xpb[cb][:]
                            base = xt.offset + (j - 1)
                        xov = bass.AP(
                            xt.tensor, base,
                            [list(xt.ap[0]), [PADH, KS], [PADH, nr], [1, W]],
                        )
                        wsl = wt[:, j * KS]
                        wov = bass.AP(
                            wsl.tensor, wsl.offset,
                            [list(wsl.ap[0]), [R * W, KS], [W, nr], [1, W]],
                        )
                        nc.vector.tensor_mul(t[:, :, :nr], xov, wov)
                        for h0, h1 in halves:
                            for i in range(KS):
                                nc.tensor.matmul(
                                    ps[:, h0:h1],
                                    ident[:],
                                    t[:, i, h0:h1],
                                    start=(jn == 0 and i == 0),
                                    stop=(jn == KS - 1 and i == KS - 1),
                                )
                    # evacuate per PSUM half (lo closes before hi), cast to
                    # bf16; host upcasts the output
                    o = ostp.tile([128, R, W], bf16, name="o", tag="o")
                    q = nc.sync if cb % 2 == 0 else nc.scalar
                    for h0, h1 in halves:
                        nc.scalar.copy(o[:, h0:h1], ps[:, h0:h1])
                        q.dma_start(
                            out[b, cb * 128:(cb + 1) * 128,
                                r0 + h0:r0 + h1, :],
                            o[:, h0:h1],
                        )
    nc.compile()
    return nc


def _build_nc_v5():
    """Channel-block-merged products via per-j shifted x copies (v5).

    The ISA caps engine APs at 3 free dims, which blocks batching the 4
    channel blocks into one DVE instruction as long as x rows live in
    68-wide padded form ((row, col) then needs its own two dims). Fix: the
    host supplies FIVE shifted copies of the padded input, one per
    horizontal tap j, each with contiguous 64-wide rows. (row, col) then
    collapses into one contiguous dim, and one tensor_mul per (chunk, j)
    covers all 4 channel blocks: free dims [cb=4, i=5, rows*cols], the
    weight operand broadcasting over cb with stride 0. 80 product instrs
    instead of 160 -> half the per-instr init/seq overhead and half the
    semaphore traffic on the critical DVE queue. Every copy is 4B-aligned,
    so the odd-j shifted-copy trick disappears too.

    Costs: x HBM traffic rises to 5 64-wide copies with an 8-row chunk
    halo (63MB/core; 124MB total, still well under the DMA budget) and
    chunks shrink to 8 rows (PSUM: 1 bank per channel block, 8 in
    flight).
    """
    import concourse.bass as bass
    import concourse.bacc as bacc
    import concourse.tile as tile
    from concourse import mybir

    f32 = mybir.dt.float32
    bf16 = mybir.dt.bfloat16
    R = 8                # output rows per chunk (PSUM: 512 f32 = 1 bank/cb)
    NCH = H // R         # 8 chunks per sample
    HALO4 = (R + KS - 1) * W   # 768: 12 rows of 64 in a shifted copy
    FRC = R * W          # 512 contiguous (row, col) elems per (cb, i)

    nc = bacc.Bacc("TRN2", target_bir_lowering=False, debug=False, num_devices=NCORES)
    # input: [S, C, j, padded_row, 64] -- five horizontally-shifted copies
    x_in = nc.declare_dram_parameter(
        "input", [S, C, KS, H + 2 * PAD, W], bf16, isOutput=False
    )
    w_in = nc.declare_dram_parameter("weight", [S, CW, KK, H, W], bf16, isOutput=False)
    id_in = nc.declare_dram_parameter("ident", [128, 128], bf16, isOutput=False)
    out = nc.declare_dram_parameter("out", [S, C, H, W], bf16, isOutput=True)

    with tile.TileContext(nc) as tc:
        with (
            tc.tile_pool(name="const", bufs=1) as constp,
            tc.tile_pool(name="xp", bufs=2) as xpp,
            tc.tile_pool(name="wt", bufs=2) as wtp,
            tc.tile_pool(name="tmp", bufs=3) as tmpp,
            tc.tile_pool(name="ts", bufs=2) as tsp,
            tc.tile_pool(name="ost", bufs=4) as ostp,
            tc.tile_pool(name="ps", bufs=8, space="PSUM") as psp,
        ):
            ident = constp.tile([128, 128], bf16)
            nc.scalar.dma_start(ident[:], id_in[:])

            chunks = [(b, k) for b in range(S) for k in range(NCH)]
            for ci, (b, k) in enumerate(chunks):
                first = ci == 0
                last = ci == len(chunks) - 1
                r0 = k * R

                # x5t[p, j, cb, row*col]: one chunk's halo rows of all five
                # shifted copies, rows contiguous per (j, cb)
                x5t = xpp.tile([128, KS, NBLK, HALO4], bf16, tag="x", name="x5t")
                wt = wtp.tile([128, KK, R, W], bf16, name="wt", tag="wt")

                def load_x(cb, j=None):
                    csl = slice(cb * 128, (cb + 1) * 128)
                    q = nc.sync if cb % 2 == 0 else nc.scalar
                    if j is None:
                        q.dma_start(
                            x5t[:, :, cb], x_in[b, csl, :, r0:r0 + R + KS - 1, :]
                        )
                    else:
                        q.dma_start(
                            x5t[:, j, cb], x_in[b, csl, j, r0:r0 + R + KS - 1, :]
                        )

                def load_w(j):
                    wsrc = w_in[b, :, j * KS:(j + 1) * KS, r0:r0 + R, :]
                    jb = slice(j * KS, (j + 1) * KS)
                    nc.sync.dma_start(wt[0:64, jb], wsrc)
                    nc.scalar.dma_start(wt[64:128, jb], wsrc)

                if first:
                    # consumption order: cb0's j0 slice + j0 weights first
                    load_x(0, 0)
                    load_w(0)
                    for j in range(1, KS):
                        load_x(0, j)
                        load_w(j)
                    for cb in range(1, NBLK):
                        load_x(cb)
                else:
                    for cb in range(NBLK):
                        load_x(cb)
                        if cb == 0:
                            for j in range(KS):
                                load_w(j)

                ps = [
                    psp.tile([128, R, W], f32, name="ps", tag="ps")
                    for _ in range(NBLK)
                ]

                def product_merged(j):
                    t = tmpp.tile([128, NBLK, KS, FRC], bf16, name="t", tag="t")
                    xov = bass.AP(
                        x5t.tensor,
                        x5t.offset + j * NBLK * HALO4,
                        [list(x5t.ap[0]), [HALO4, NBLK], [W, KS], [1, FRC]],
                    )
                    wsl = wt[:, j * KS]
                    wov = bass.AP(
                        wsl.tensor, wsl.offset,
                        [list(wsl.ap[0]), [0, NBLK], [FRC, KS], [1, FRC]],
                    )
                    nc.vector.tensor_mul(t[:], xov, wov)
                    return t

                def product_cb(j, cb):
                    t = tsp.tile([128, KS, FRC], bf16, name="tsg", tag="ts")
                    xov = bass.AP(
                        x5t.tensor,
                        x5t.offset + (j * NBLK + cb) * HALO4,
                        [list(x5t.ap[0]), [W, KS], [1, FRC]],
                    )
                    wsl = wt[:, j * KS]
                    wov = bass.AP(
                        wsl.tensor, wsl.offset,
                        [list(wsl.ap[0]), [FRC, KS], [1, FRC]],
                    )
                    nc.vector.tensor_mul(t[:], xov, wov)
                    return t

                def mm(mv, cb, j, i):
                    nc.tensor.matmul(
                        ps[cb][:],
                        ident[:],
                        mv[:, i],
                        start=(j == 0 and i == 0),
                        stop=(j == KS - 1 and i == KS - 1),
                    )

                def store(cb):
                    o = ostp.tile([128, R, W], bf16, name="o", tag="o")
                    nc.scalar.copy(o[:], ps[cb][:])
                    q = nc.sync if cb % 2 == 0 else nc.scalar
                    q.dma_start(
                        out[b, cb * 128:(cb + 1) * 128, r0:r0 + R, :], o[:]
                    )

                for j in range(KS):
                    split = (first and j == 0) or (last and j == KS - 1)
                    if split:
                        for cb in range(NBLK):
                            t = product_cb(j, cb)
                            for i in range(KS):
                                mm(t, cb, j, i)
                            if j == KS - 1:
                                store(cb)
                    else:
                        t = product_merged(j)
                        for cb in range(NBLK):
                            tcb = t[:, cb]
                            for i in range(KS):
                                mm(tcb, cb, j, i)
                            if j == KS - 1:
                                store(cb)
    nc.compile()
    return nc


def _build_nc_v6():
    """Channel-block-merged products from three 66-wide shifted copies.

    Same idea as v5 (collapse (row, col) into one contiguous AP dim so one
    tensor_mul per (chunk, j) covers all 4 channel blocks, weight operand
    broadcast over cb with stride 0), but with the x-traffic blow-up
    fixed: instead of five 64-wide copies, THREE 66-wide copies serve the
    five horizontal taps at even in-row offsets (j0->copy0+0, j1->copy1+0,
    j2->copy2+0, j3->copy1+2, j4->copy2+2), keeping every operand
    4B-aligned for the DVE 2x mode. Rows are 66 wide, so each product
    carries 2 garbage columns per row (3.1% DVE tax) that the PE moving
    APs simply skip; PSUM tiles stay 64-wide and bank-aligned. x HBM
    traffic is 3*66/(2*68) of the old two-copy scheme (~+10MB/core),
    total DMA ~95MB -- comfortably off the critical path, unlike v5.
    40 merged product instrs instead of 160.
    """
    import concourse.bass as bass
    import concourse.bacc as bacc
    import concourse.tile as tile
    from concourse import mybir

    f32 = mybir.dt.float32
    bf16 = mybir.dt.bfloat16
    R = 16               # output rows per chunk
    NCH = H // R         # 4 chunks per sample
    HB = R // 2          # rows per PSUM half
    WC = 66              # copy row width
    FRC = R * WC         # 1056 contiguous (row, col) elems per (cb, i)
    HALOC = (R + KS - 1) * WC  # 1320: 20 rows of 66 per copy
    NCP = 3
    # j -> (copy, even in-row offset)
    JMAP = {0: (0, 0), 1: (1, 0), 2: (2, 0), 3: (1, 2), 4: (2, 2)}

    nc = bacc.Bacc("TRN2", target_bir_lowering=False, debug=False, num_devices=NCORES)
    # input: three 66-wide shifted copies of the zero-padded image
    x_in = nc.declare_dram_parameter(
        "input", [S, C, NCP, H + 2 * PAD, WC], bf16, isOutput=False
    )
    # weight: taps j-major, rows padded to 66 cols (last 2 cols zero)
    w_in = nc.declare_dram_parameter(
        "weight", [S, CW, KK, H, WC], bf16, isOutput=False
    )
    id_in = nc.declare_dram_parameter("ident", [128, 128], bf16, isOutput=False)
    out = nc.declare_dram_parameter("out", [S, C, H, W], bf16, isOutput=True)

    with tile.TileContext(nc) as tc:
        with (
            tc.tile_pool(name="const", bufs=1) as constp,
            tc.tile_pool(name="xp", bufs=2) as xpp,
            tc.tile_pool(name="wt", bufs=1) as wtp,
            tc.tile_pool(name="tmp", bufs=2) as tmpp,
            tc.tile_pool(name="ost", bufs=2) as ostp,
            tc.tile_pool(name="ps", bufs=4, space="PSUM") as psp,
        ):
            ident = constp.tile([128, 128], bf16)
            nc.scalar.dma_start(ident[:], id_in[:])

            chunks = [(b, k) for b in range(S) for k in range(NCH)]
            for ci, (b, k) in enumerate(chunks):
                first = ci == 0
                last = ci == len(chunks) - 1
                r0 = k * R

                # per-copy tiles [p, cb, row*col], +8 elems of slack for the
                # offset-2 products' tail overrun into garbage
                cps = [
                    xpp.tile([128, NBLK * HALOC + 8], bf16, tag=f"cp{c}",
                             name=f"cp{c}")
                    for c in range(NCP)
                ]
                wts = [
                    wtp.tile([128, KS, FRC], bf16, tag=f"wt{j}", name=f"wt{j}")
                    for j in range(KS)
                ]

                def load_cp(cp, cb0=0, ncb=NBLK):
                    q = (nc.sync, nc.scalar, nc.sync)[cp]
                    dst = cps[cp][:, cb0 * HALOC:(cb0 + ncb) * HALOC]
                    dst = bass.AP(
                        dst.tensor, dst.offset,
                        [list(dst.ap[0]), [HALOC, ncb], [1, HALOC]],
                    )
                    src = x_in[b, :, cp, r0:r0 + R + KS - 1, :]
                    sap = bass.AP(
                        src.tensor, src.offset + cb0 * 128 * NCP * PADH * WC,
                        [[NCP * PADH * WC, 128], [128 * NCP * PADH * WC, ncb],
                         [1, HALOC]],
                    )
                    q.dma_start(dst, sap)

                def load_w(j):
                    wsrc = w_in[b, :, j * KS:(j + 1) * KS, r0:r0 + R, :]
                    nc.sync.dma_start(wts[j][0:64], wsrc)
                    nc.scalar.dma_start(wts[j][64:128], wsrc)

                if first:
                    # cb0's copy-0 slice + j0 weights gate the first product
                    load_cp(0, 0, 1)
                    load_w(0)
                    load_cp(0, 1, 3)
                    load_cp(1)
                    load_w(1)
                    load_cp(2)
                    for j in range(2, KS):
                        load_w(j)
                else:
                    for cp in range(NCP):
                        load_cp(cp)
                    for j in range(KS):
                        load_w(j)

                ps = [
                    psp.tile([128, R, W], f32, name="ps", tag="ps")
                    for _ in range(NBLK)
                ]

                for j in range(KS):
                    cp, ofs = JMAP[j]
                    xt = cps[cp]
                    t = tmpp.tile([128, NBLK, KS, FRC], bf16, name="t", tag="t")
                    wsl = wts[j][:]
                    # fill/drain chunks: issue j0/j4 per channel block (into
                    # slices of the same tile) so the first product waits on
                    # 1/4 of the x bytes and the tail drains 10 matmuls, not
                    # 40
                    split = (first and j == 0) or (last and j == KS - 1)
                    if split:
                        for cb in range(NBLK):
                            xov = bass.AP(
                                xt.tensor, xt.offset + cb * HALOC + ofs,
                                [list(xt.ap[0]), [WC, KS], [1, FRC]],
                            )
                            wov = bass.AP(
                                wsl.tensor, wsl.offset,
                                [list(wsl.ap[0]), [FRC, KS], [1, FRC]],
                            )
                            nc.vector.tensor_mul(t[:, cb], xov, wov)
                    else:
                        xov = bass.AP(
                            xt.tensor, xt.offset + ofs,
                            [list(xt.ap[0]), [HALOC, NBLK], [WC, KS], [1, FRC]],
                        )
                        wov = bass.AP(
                            wsl.tensor, wsl.offset,
                            [list(wsl.ap[0]), [0, NBLK], [FRC, KS], [1, FRC]],
                        )
                        nc.vector.tensor_mul(t[:], xov, wov)
                    for cb in range(NBLK):
                        for half in range(2):
                            for i in range(KS):
                                mv = bass.AP(
                                    t.tensor,
                                    t.offset + (cb * KS + i) * FRC
                                    + half * HB * WC,
                                    [list(t.ap[0]), [WC, HB], [1, W]],
                                )
                                nc.tensor.matmul(
                                    ps[cb][:, half * HB:(half + 1) * HB],
                                    ident[:],
                                    mv,
                                    start=(j == 0 and i == 0),
                                    stop=(j == KS - 1 and i == KS - 1),
                                )
                        if j == KS - 1:
                            o = ostp.tile([128, R, W], bf16, name="o", tag="o")
                            nc.scalar.copy(o[:], ps[cb][:])
                            q = nc.sync if cb % 2 == 0 else nc.scalar
                            q.dma_start(
                                out[b, cb * 128:(cb + 1) * 128, r0:r0 + R, :],
                                o[:],
                            )
    nc.compile()
    return nc


def _build_nc_v7():
    """Samples-on-partitions layout (v7).

    Partition p = (sample b = p//64, cw = p%64) instead of a 128-channel
    block. Wins:
    - The 128 weight partition rows are all DISTINCT (two samples' cw
      rows), so the weight partition-duplication double-read disappears:
      w traffic halves to 26MB/core.
    - The share-group dim g (8 groups whose channels reuse the same cw
      weights) moves to the free dims, so ONE tensor_mul per (chunk, tap
      j) covers the whole chunk: free dims [g=8, i=5, row*col] -- exactly
      the ISA's 3-free-dim cap, with the weight operand broadcast over g
      by stride 0 and (row, col) collapsed via host-side per-j shifted
      64-wide x copies (all offsets 0 -> always 4B-aligned, no odd-shift
      copy). 40 product instrs of FD 20480: ~432us, near the 427us DVE
      floor, vs ~453us for the 160-instr per-channel-block structure.
    - Per-j x and w tiles are consumed by that one instr early in the
      chunk, so they single-buffer; their next-chunk reloads self-stagger
      across the chunk (WAR on the DVE product), smoothing DMA demand.

    8-row chunks: per (chunk, g) PSUM tile [128, 8, 64] f32 = exactly one
    bank, 8 groups = all 8 banks; evacuation per g frees its bank long
    before the next chunk's accumulation reaches it. Products stay bf16
    with fp32 PSUM accumulation; output bf16 (host upcasts).
    """
    import concourse.bass as bass
    import concourse.bacc as bacc
    import concourse.tile as tile
    from concourse import mybir

    f32 = mybir.dt.float32
    bf16 = mybir.dt.bfloat16
    G = C // CW          # 8 share groups
    R = 8                # output rows per chunk (1 PSUM bank per group)
    NCH = H // R         # 8 chunks per sample-pair
    HR = R + KS - 1      # 12 halo rows per chunk
    FRC = R * W          # 512 contiguous (row, col) elems
    FH = HR * W          # 768 halo elems per (j, g)

    nc = bacc.Bacc("TRN2", target_bir_lowering=False, debug=False, num_devices=NCORES)
    # input: [p=(b,cw), g, j, padded_row, 64] -- per-j shifted copies
    x_in = nc.declare_dram_parameter(
        "input", [128, G, KS, H + 2 * PAD, W], bf16, isOutput=False
    )
    # weight: [p=(b,cw), j, i, row, col] -- j-major taps, no duplication
    w_in = nc.declare_dram_parameter(
        "weight", [128, KS, KS, H, W], bf16, isOutput=False
    )
    id_in = nc.declare_dram_parameter("ident", [128, 128], bf16, isOutput=False)
    out = nc.declare_dram_parameter("out", [S, C, H, W], bf16, isOutput=True)

    with tile.TileContext(nc) as tc:
        with (
            tc.tile_pool(name="const", bufs=1) as constp,
            tc.tile_pool(name="xp", bufs=1) as xpp,
            tc.tile_pool(name="wt", bufs=1) as wtp,
            tc.tile_pool(name="tmp", bufs=2) as tmpp,
            tc.tile_pool(name="ost", bufs=8) as ostp,
            tc.tile_pool(name="ps", bufs=8, space="PSUM") as psp,
        ):
            ident = constp.tile([128, 128], bf16)
            nc.scalar.dma_start(ident[:], id_in[:])

            # Queue discipline: the sync queue carries ONLY loads, the
            # scalar queue ONLY evacuation copies + store triggers. Both
            # evacuations (waiting on PE's chunk-end stop matmuls) and
            # store triggers resolve late in each chunk; placing any load
            # trigger behind them on the same FIFO queue head-of-line
            # blocks the next chunk's x/w by ~10-15us.
            for ci in range(NCH):
                first = ci == 0
                last = ci == NCH - 1
                r0 = ci * R

                HG = G // 2   # groups per product half
                xts = [
                    xpp.tile([128, G, FH], bf16, tag=f"x{j}", name=f"x{j}")
                    for j in range(KS)
                ]
                wts = [
                    wtp.tile([128, KS, FRC], bf16, tag=f"w{j}", name=f"w{j}")
                    for j in range(KS)
                ]

                def load_x(j, h):
                    # per-half loads so each half-product waits only on its
                    # own DMA
                    g0 = h * HG
                    nc.sync.dma_start(
                        xts[j][:, g0:g0 + HG],
                        x_in[:, g0:g0 + HG, j, r0:r0 + HR, :],
                    )

                def load_w(j):
                    nc.sync.dma_start(wts[j][:], w_in[:, j, :, r0:r0 + R, :])

                if first:
                    load_x(0, 0)
                    load_w(0)
                    load_x(0, 1)
                    for j in range(1, KS):
                        load_x(j, 0)
                        load_w(j)
                        load_x(j, 1)
                else:
                    for j in range(KS):
                        load_x(j, 0)
                        load_x(j, 1)
                        load_w(j)

                ps = [
                    psp.tile([128, R, W], f32, name="ps", tag="ps")
                    for _ in range(G)
                ]

                def product(j, h):
                    # half-group product: staggers PE consumption, PSUM
                    # completion and evacuation across the chunk instead of
                    # bunching them at the chunk boundary
                    t = tmpp.tile([128, HG, KS, FRC], bf16, name="t",
                                  tag=f"t{h}")
                    xt = xts[j]
                    xov = bass.AP(
                        xt.tensor, xt.offset + h * HG * FH,
                        [list(xt.ap[0]), [FH, HG], [W, KS], [1, FRC]],
                    )
                    wsl = wts[j][:]
                    wov = bass.AP(
                        wsl.tensor, wsl.offset,
                        [list(wsl.ap[0]), [0, HG], [FRC, KS], [1, FRC]],
                    )
                    nc.vector.tensor_mul(t[:], xov, wov)
                    return t

                def store(g):
                    o = ostp.tile([128, R, W], bf16, name="o", tag="o")
                    nc.scalar.copy(o[:], ps[g][:])
                    # p = (b, cw) -> out[b, g*64+cw, r0:r0+R, :]
                    osl = out[0, g * CW, r0]
                    nc.scalar.dma_start(
                        bass.AP(
                            osl.tensor, osl.offset,
                            [[C * H * W, S], [H * W, CW], [1, FRC]],
                        ),
                        o[:],
                    )

                for j in range(KS):
                    for h in range(2):
                        t = product(j, h)
                        for g4 in range(HG):
                            g = h * HG + g4
                            for i in range(KS):
                                nc.tensor.matmul(
                                    ps[g][:],
                                    ident[:],
                                    t[:, g4, i],
                                    start=(j == 0 and i == 0),
                                    stop=(j == KS - 1 and i == KS - 1),
                                )
                            if j == KS - 1:
                                store(g)
    nc.compile()
    return nc


def _variant():
    return os.environ.get("BASS_KERNEL_VARIANT", "v3")


def _get_nc():
    v = _variant()
    if v not in _STATE:
        if v == "v7":
            _STATE[v] = _build_nc_v7()
        elif v == "v6":
            _STATE[v] = _build_nc_v6()
        elif v == "v5":
            _STATE[v] = _build_nc_v5()
        elif v == "v3":
            _STATE[v] = _build_nc_v3()
        elif v == "bf16":
            _STATE[v] = _build_nc_bf16()
        else:
            _STATE[v] = _build_nc()
    return _STATE[v]


def run(input, weight, trace=False):
    """Run on 8 NeuronCores; returns (output, BassKernelResults)."""
    from concourse.bass_utils import run_bass_kernel_spmd

    assert input.shape == (B, C, H, W), input.shape
    assert weight.shape == (B, CW, KK, H * W), weight.shape
    if _variant() == "v7":
        import ml_dtypes

        dt = ml_dtypes.bfloat16
        # Partition = (sample, cw): per core, [128, g, j, row, 64] with
        # per-j shifted 64-wide copies of the zero-padded image.
        G = C // CW
        xpad = np.zeros((B, C, PADH, PADH), dtype=dt)
        xpad[:, :, PAD:PAD + H, PAD:PAD + W] = np.asarray(
            input, dtype=np.float32
        ).astype(dt)
        xv = xpad.reshape(NCORES, S, G, CW, PADH, PADH)
        xv = np.ascontiguousarray(xv.transpose(0, 1, 3, 2, 4, 5))
        xv = xv.reshape(NCORES, 128, G, PADH, PADH)
        inp = np.empty((NCORES, 128, G, 5, PADH, W), dtype=dt)
        for j in range(5):
            inp[:, :, :, j] = xv[..., j:j + W]
    elif _variant() == "v6":
        import ml_dtypes

        dt = ml_dtypes.bfloat16
        # Three 66-wide shifted copies of the zero-padded input with
        # contiguous rows: the five horizontal taps read them at even
        # in-row offsets, keeping the DVE 2x-mode alignment.
        xpad = np.zeros((B, C, PADH, PADH), dtype=dt)
        xpad[:, :, PAD:PAD + H, PAD:PAD + W] = np.asarray(
            input, dtype=np.float32
        ).astype(dt)
        inp = np.empty((B, C, 3, PADH, 66), dtype=dt)
        for c in range(3):
            inp[:, :, c] = xpad[:, :, :, c:c + 66]
    elif _variant() == "v5":
        import ml_dtypes

        dt = ml_dtypes.bfloat16
        # Five horizontally-shifted copies of the zero-padded input, each
        # with contiguous 64-wide rows: x5[b, c, j, r, :] = xpad[b, c, r,
        # j:j+64]. Lets the kernel collapse (row, col) into one AP dim.
        xpad = np.zeros((B, C, PADH, PADH), dtype=dt)
        xpad[:, :, PAD:PAD + H, PAD:PAD + W] = np.asarray(
            input, dtype=np.float32
        ).astype(dt)
        inp = np.empty((B, C, 5, PADH, W), dtype=dt)
        for j in range(5):
            inp[:, :, j] = xpad[:, :, :, j:j + W]
    elif _variant() in ("bf16", "v3"):
        import ml_dtypes

        dt = ml_dtypes.bfloat16
        # Pre-pad on the host: [B, C, 68*68+1] with zero borders, so the
        # kernel's two shifted SBUF copies are fully contiguous DMAs.
        inp = np.zeros((B, C, PADH * PADH + 1), dtype=dt)
        view = inp[:, :, :PADH * PADH].reshape(B, C, PADH, PADH)
        view[:, :, PAD:PAD + H, PAD:PAD + W] = np.asarray(
            input, dtype=np.float32
        ).astype(dt)
    else:
        dt = np.float32
        inp = np.ascontiguousarray(np.asarray(input, dtype=np.float32))
    wgt = np.asarray(weight, dtype=np.float32).astype(dt).reshape(B, CW, KK, H, W)
    if _variant() in ("bf16", "v3", "v5", "v6", "v7"):
        # j-major tap order: slot j*5+i holds original tap i*5+j, so each
        # horizontal shift's 5 vertical taps are contiguous in SBUF
        wgt = wgt.reshape(B, CW, KS, KS, H, W).transpose(0, 1, 3, 2, 4, 5)
        wgt = wgt.reshape(B, CW, KK, H, W)
    if _variant() == "v6":
        # rows padded to 66 cols (zeros) to match the 66-wide x copies
        w66 = np.zeros((B, CW, KK, H, 66), dtype=dt)
        w66[..., :W] = wgt
        wgt = w66
    if _variant() == "v7":
        # [core, p=(b,cw), j, i, row, col]
        wgt = wgt.reshape(NCORES, 128, KS, KS, H, W)
    wgt = np.ascontiguousarray(wgt)
    ident = np.eye(128, dtype=dt)

    nc = _get_nc()
    core_ids = list(range(NCORES))
    if _variant() == "v7":
        in_maps = [
            {"input": inp[c], "weight": wgt[c], "ident": ident}
            for c in core_ids
        ]
    else:
        in_maps = [
            {
                "input": inp[c * S:(c + 1) * S],
                "weight": wgt[c * S:(c + 1) * S],
                "ident": ident,
            }
            for c in core_ids
        ]
    res = run_bass_kernel_spmd(nc, in_maps, core_ids, trace=trace)
    outp = np.concatenate([res.results[c]["out"] for c in core_ids], axis=0)
    outp = np.ascontiguousarray(outp.astype(np.float32))
    return outp, res


def kernel(input, weight):
    trace = bool(int(os.environ.get("BASS_KERNEL_TRACE", "0")))
    outp, _ = run(input, weight, trace=trace)
    return outp



# revision 41
# speedup vs baseline: 1.2071x; 1.1577x over previous
"""Trainium2 Bass kernel for nn_Aggregation (involution-style local aggregation).

out[b, g*64+cw, ho, wo] = sum_{i,j in 5x5} xpad[b, g*64+cw, ho+i, wo+j]
                          * weight[b, cw, i*5+j, ho*64+wo]

Data-parallel over batch: 16 samples -> 8 NeuronCores, 2 samples/core.
Per core:
  - DVE computes the 25 shifted elementwise products (batched 5 window
    shifts per tensor_tensor via an overlapping access pattern),
  - TensorE accumulates them into PSUM with identity-stationary matmuls
    (1 cycle/row),
  - ScalarE evacuates PSUM -> SBUF, DMA writes back.
"""

import os
import sys

import numpy as np

sys.path.insert(0, "/opt/trn_rl_repo")

# Problem constants (hardcoded per contract)
B, C, H, W = 16, 512, 64, 64
CW, KK, KS = 64, 25, 5
PAD = 2
NCORES = 8
S = B // NCORES          # samples per core = 2
PADH = H + 2 * PAD       # 68
NBLK = C // 128          # 4 channel blocks of 128 (each = 2 share-groups)
ROWS = 8                 # output rows per chunk
CHUNK = ROWS * W         # 512 positions = 1 PSUM bank of fp32
NCHUNK = H // ROWS       # 8 chunks per sample

_STATE = {}


def _build_nc():
    import concourse.bass as bass
    import concourse.bacc as bacc
    import concourse.tile as tile
    from concourse import mybir

    f32 = mybir.dt.float32
    f32r = mybir.dt.float32r

    nc = bacc.Bacc("TRN2", target_bir_lowering=False, debug=False, num_devices=NCORES)
    x_in = nc.declare_dram_parameter("input", [S, C, H, W], f32, isOutput=False)
    w_in = nc.declare_dram_parameter("weight", [S, CW, KK, H, W], f32, isOutput=False)
    id_in = nc.declare_dram_parameter("ident", [128, 128], f32, isOutput=False)
    out = nc.declare_dram_parameter("out", [S, C, H, W], f32, isOutput=True)

    with tile.TileContext(nc) as tc:
        with (
            tc.tile_pool(name="const", bufs=1) as constp,
            tc.tile_pool(name="xp", bufs=1) as xpp,
            tc.tile_pool(name="wt", bufs=2) as wtp,
            tc.tile_pool(name="tmp", bufs=2) as tmpp,
            tc.tile_pool(name="ost", bufs=2) as ostp,
            tc.tile_pool(name="ps", bufs=4, space="PSUM") as psp,
        ):
            ident = constp.tile([128, 128], f32)
            nc.sync.dma_start(ident[:], id_in[:])

            # Persistent padded-input tiles, one per channel block. Borders
            # are zeroed once; only the interior is rewritten per sample.
            xp = []
            for cb in range(NBLK):
                t = xpp.tile([128, PADH, PADH], f32, tag=f"xp{cb}")
                nc.vector.memset(t[:], 0.0)
                xp.append(t)

            for b in range(S):
                for cb in range(NBLK):
                    nc.sync.dma_start(
                        xp[cb][:, PAD:PAD + H, PAD:PAD + W],
                        x_in[b, cb * 128:(cb + 1) * 128],
                    )
                for k in range(NCHUNK):
                    wt = wtp.tile([128, KK, ROWS, W], f32)
                    # weight rows for this chunk on partitions 0..63, then
                    # duplicated to 64..127 (channel blocks span 2 groups
                    # sharing the same cw range).
                    nc.sync.dma_start(
                        wt[0:64], w_in[b, :, :, k * ROWS:(k + 1) * ROWS, :]
                    )
                    nc.sync.dma_start(wt[64:128], wt[0:64])
                    for cb in range(NBLK):
                        ps = psp.tile([128, ROWS, W], f32)
                        for j in range(KS):
                            t = tmpp.tile([128, KS, ROWS, W], f32)
                            # x window, batched over the 5 vertical shifts i:
                            # dims (i:5 @ PADH, r:ROWS @ PADH, c:W @ 1),
                            # base offset = (k*ROWS)*PADH + j
                            sl = xp[cb][:, k * ROWS:k * ROWS + ROWS, j:j + W]
                            xov = bass.AP(
                                sl.tensor, sl.offset,
                                [list(sl.ap[0]), [PADH, KS], [PADH, ROWS], [1, W]],
                            )
                            # weight idx = i*5+j for i in 0..5:
                            # offset j*ROWS*W, stride 5*ROWS*W over i
                            wsl = wt[:, j]
                            wov = bass.AP(
                                wsl.tensor, wsl.offset,
                                [list(wsl.ap[0]), [KS * ROWS * W, KS], [W, ROWS], [1, W]],
                            )
                            nc.vector.tensor_mul(t[:], xov, wov)
                            for i in range(KS):
                                idx = i * KS + j
                                nc.tensor.matmul(
                                    ps[:],
                                    ident[:].bitcast(f32r),
                                    t[:, i].bitcast(f32r),
                                    start=(j == 0 and i == 0),
                                    stop=(j == KS - 1 and i == KS - 1),
                                )
                        o = ostp.tile([128, ROWS, W], f32)
                        nc.scalar.copy(o[:], ps[:])
                        nc.sync.dma_start(
                            out[b, cb * 128:(cb + 1) * 128, k * ROWS:(k + 1) * ROWS, :],
                            o[:],
                        )
    nc.compile()
    return nc


def _build_nc_bf16():
    """bf16-products variant (measured ~473 us/core on 8 cores).

    - DVE tensor_tensor runs in 2x_1P mode (2 elem/cycle/lane): every operand
      is bf16, innermost stride 1, 4B-aligned. Odd horizontal shifts j break
      4B alignment, so a second copy of the input, stored shifted by one
      element, serves the odd-j windows.
    - The host supplies the input pre-padded (zero borders, 68x68 per image)
      and pre-cast to bf16 as [S, C, 68*68+1]; the kernel streams it in
      per-chunk row-halo tiles (20 padded rows), fully contiguous transfers
      for both shifted copies.
    - Each tensor_tensor batches the 5 vertical taps of one horizontal shift
      via an overlapping access pattern (free size 5*16*64 = 5120).
    - Products are bf16; the 25-tap accumulation stays exact in fp32 PSUM via
      identity-stationary matmuls (1 cycle/row bf16; identity loads hide
      under the matmul stream). ScalarE evacuates PSUM -> SBUF -> DMA out.
    """
    import concourse.bass as bass
    import concourse.bacc as bacc
    import concourse.tile as tile
    from concourse import mybir

    f32 = mybir.dt.float32
    bf16 = mybir.dt.bfloat16
    NPAD = PADH * PADH   # 4624
    R = 16               # output rows per chunk
    NCH = H // R         # 4 chunks per sample
    HB = R // 2          # rows per PSUM half (512 fp32 = one bank)
    HALO = (R + KS - 1) * PADH  # 20 padded rows = 1360 elements

    nc = bacc.Bacc("TRN2", target_bir_lowering=False, debug=False, num_devices=NCORES)
    x_in = nc.declare_dram_parameter("input", [S, C, NPAD + 1], bf16, isOutput=False)
    w_in = nc.declare_dram_parameter("weight", [S, CW, KK, H, W], bf16, isOutput=False)
    id_in = nc.declare_dram_parameter("ident", [128, 128], bf16, isOutput=False)
    out = nc.declare_dram_parameter("out", [S, C, H, W], f32, isOutput=True)

    with tile.TileContext(nc) as tc:
        with (
            tc.tile_pool(name="const", bufs=1) as constp,
            tc.tile_pool(name="xp", bufs=2) as xpp,
            tc.tile_pool(name="wt", bufs=2) as wtp,
            tc.tile_pool(name="tmp", bufs=5) as tmpp,
            tc.tile_pool(name="ost", bufs=3) as ostp,
            tc.tile_pool(name="ps", bufs=4, space="PSUM") as psp,
        ):
            ident = constp.tile([128, 128], bf16)
            nc.scalar.dma_start(ident[:], id_in[:])

            for b in range(S):
                for k in range(NCH):
                    row0 = k * R * PADH
                    # Two parallel HBM reads of the same weight rows replace
                    # the former SBUF->SBUF partition-duplication DMA, which
                    # serialized behind the x transfers on its FIFO queue and
                    # gated the first products of every chunk. x loads split
                    # across the two HWDGE queues likewise. The very first
                    # chunk streams everything in exact consumption order
                    # (cb0's x, then the 5 tap blocks just-in-time, then the
                    # remaining channel blocks' x) so the DVE product stream
                    # starts ~10us in and never stalls.
                    wsrc = w_in[b, :, :, k * R:(k + 1) * R, :]
                    wt = wtp.tile([128, KK, R, W], bf16, name="wt", tag="wt")
                    xpa, xpb = [], []
                    for cb in range(NBLK):
                        csl = slice(cb * 128, (cb + 1) * 128)
                        ta = xpp.tile([128, HALO], bf16, tag=f"xpa{cb}", name=f"xpa{cb}")
                        tb = xpp.tile([128, HALO], bf16, tag=f"xpb{cb}", name=f"xpb{cb}")
                        xpa.append(ta)
                        xpb.append(tb)
                    first = (b == 0 and k == 0)
                    def load_x(cb):
                        csl = slice(cb * 128, (cb + 1) * 128)
                        nc.sync.dma_start(xpa[cb][:], x_in[b, csl, row0:row0 + HALO])
                        nc.scalar.dma_start(
                            xpb[cb][:], x_in[b, csl, row0 + 1:row0 + 1 + HALO]
                        )
                    if first:
                        load_x(0)
                        for j in range(KS):
                            jb = slice(j * KS, (j + 1) * KS)
                            nc.sync.dma_start(wt[0:64, jb], wsrc[:, jb])
                            nc.scalar.dma_start(wt[64:128, jb], wsrc[:, jb])
                        for cb in range(1, NBLK):
                            load_x(cb)
                    else:
                        for cb in range(NBLK):
                            load_x(cb)
                            if cb == 0:
                                nc.sync.dma_start(wt[0:64], wsrc)
                                nc.scalar.dma_start(wt[64:128], wsrc)
                    for cb in range(NBLK):
                        ps = psp.tile([128, R, W], f32, name="ps", tag="ps")
                        for j in range(KS):
                            t = tmpp.tile([128, KS, R, W], bf16, name="t", tag="t")
                            if j % 2 == 0:
                                xt = xpa[cb][:]
                                base = xt.offset + j
                            else:
                                xt = xpb[cb][:]
                                base = xt.offset + (j - 1)
                            xov = bass.AP(
                                xt.tensor, base,
                                [list(xt.ap[0]), [PADH, KS], [PADH, R], [1, W]],
                            )
                            # weight is j-major on the host: taps for this j
                            # are the contiguous block wt[:, j*5:(j+1)*5]
                            wsl = wt[:, j * KS]
                            wov = bass.AP(
                                wsl.tensor, wsl.offset,
                                [list(wsl.ap[0]), [R * W, KS], [W, R], [1, W]],
                            )
                            nc.vector.tensor_mul(t[:], xov, wov)
                            for half in range(2):
                                for i in range(KS):
                                    nc.tensor.matmul(
                                        ps[:, half * HB:(half + 1) * HB],
                                        ident[:],
                                        t[:, i, half * HB:(half + 1) * HB],
                                        start=(j == 0 and i == 0),
                                        stop=(j == KS - 1 and i == KS - 1),
                                    )
                        # evacuate and store per PSUM half: the lo half's
                        # accumulation closes ~1us before the hi half's, so
                        # its copy and store overlap the hi half's tail
                        o = ostp.tile([128, R, W], f32, name="o", tag="o")
                        for h0, h1 in ((0, HB), (HB, R)):
                            nc.scalar.copy(o[:, h0:h1], ps[:, h0:h1])
                            nc.scalar.dma_start(
                                out[b, cb * 128:(cb + 1) * 128,
                                    k * R + h0:k * R + h1, :],
                                o[:, h0:h1],
                            )
    nc.compile()
    return nc


def _build_nc_v3():
    """Incremental variant over _build_nc_bf16.

    DVE is the bottleneck engine (the 25-tap elementwise products are
    105M bf16 elems/core at 2 elem/cycle/lane = 427us floor; nothing else
    on TRN2 can multiply two position-varying tensors: ScalarE has no
    tensor*tensor, GPSIMD's tensor_tensor is locked out of its shared SBUF
    port while DVE runs, PE needs a stationary operand; the ISA's
    3-free-dim AP cap rules out batching more taps/blocks per DVE
    instruction). Remaining wins are around the DVE stream:

    - bf16 output (host upcasts): halves store traffic.
    - The first sample's first 16 rows are processed as two 8-row
      sub-chunks, halving the data the very first product depends on
      (x halo + weight block), so the DVE starts ~6us earlier.
    - The fill-phase chunk runs its taps even-j-first (0,2,4,1,3), so the
      shifted x copy (xpb, odd j) drops out of the first dependency.
    """
    import concourse.bass as bass
    import concourse.bacc as bacc
    import concourse.tile as tile
    from concourse import mybir

    f32 = mybir.dt.float32
    bf16 = mybir.dt.bfloat16
    NPAD = PADH * PADH   # 4624
    R = 16               # output rows per chunk
    NCH = H // R         # 4 chunks per sample
    HB = R // 2          # rows per PSUM half (512 fp32 = one bank)
    HALO = (R + KS - 1) * PADH  # 20 padded rows = 1360 elements

    nc = bacc.Bacc("TRN2", target_bir_lowering=False, debug=False, num_devices=NCORES)
    x_in = nc.declare_dram_parameter("input", [S, C, NPAD + 1], bf16, isOutput=False)
    w_in = nc.declare_dram_parameter("weight", [S, CW, KK, H, W], bf16, isOutput=False)
    id_in = nc.declare_dram_parameter("ident", [128, 128], bf16, isOutput=False)
    out = nc.declare_dram_parameter("out", [S, C, H, W], bf16, isOutput=True)

    CSTR = NPAD + 1      # channel stride in padded input

    with tile.TileContext(nc) as tc:
        with (
            tc.tile_pool(name="const", bufs=1) as constp,
            tc.tile_pool(name="xp", bufs=2) as xpp,
            tc.tile_pool(name="wt", bufs=2) as wtp,
            tc.tile_pool(name="tmp", bufs=5) as tmpp,
            tc.tile_pool(name="ost", bufs=3) as ostp,
            tc.tile_pool(name="ps", bufs=4, space="PSUM") as psp,
        ):
            ident = constp.tile([128, 128], bf16)
            nc.scalar.dma_start(ident[:], id_in[:])

            # (sample, row0, nrows): the first 16-row chunk is split in two
            # so the pipeline fill waits on half the x/w bytes
            chunks = [(0, 0, 8), (0, 8, 8)]
            chunks += [(0, r, R) for r in range(R, H, R)]
            chunks += [(1, r, R) for r in range(0, H, R)]

            for ci, (b, r0, nr) in enumerate(chunks):
                first = ci == 0
                row0 = r0 * PADH
                halo = (nr + KS - 1) * PADH
                jorder = (0, 2, 4, 1, 3) if ci < 2 else (0, 1, 2, 3, 4)

                xpa, xpb = [], []
                for cb in range(NBLK):
                    xpa.append(xpp.tile([128, HALO], bf16, tag=f"xpa{cb}",
                                        name=f"xpa{cb}"))
                    xpb.append(xpp.tile([128, HALO], bf16, tag=f"xpb{cb}",
                                        name=f"xpb{cb}"))
                wt = wtp.tile([128, KK, R, W], bf16, name="wt", tag="wt")

                def load_x(cb):
                    csl = slice(cb * 128, (cb + 1) * 128)
                    nc.sync.dma_start(
                        xpa[cb][:, :halo], x_in[b, csl, row0:row0 + halo]
                    )
                    nc.scalar.dma_start(
                        xpb[cb][:, :halo], x_in[b, csl, row0 + 1:row0 + 1 + halo]
                    )

                def load_xa(cb):
                    csl = slice(cb * 128, (cb + 1) * 128)
                    nc.sync.dma_start(
                        xpa[cb][:, :halo], x_in[b, csl, row0:row0 + halo]
                    )

                def load_xb(cb):
                    csl = slice(cb * 128, (cb + 1) * 128)
                    nc.scalar.dma_start(
                        xpb[cb][:, :halo], x_in[b, csl, row0 + 1:row0 + 1 + halo]
                    )

                def load_w(j):
                    wsrc = w_in[b, :, j * KS:(j + 1) * KS, r0:r0 + nr, :]
                    jb = slice(j * KS, (j + 1) * KS)
                    nc.sync.dma_start(wt[0:64, jb, :nr], wsrc)
                    nc.scalar.dma_start(wt[64:128, jb, :nr], wsrc)

                if first:
                    # exact consumption order; even-j taps run first, so the
                    # fill needs only xpa0 + the even weight blocks upfront
                    load_xa(0)
                    for j in (0, 2, 4):
                        load_w(j)
                    load_xb(0)
                    for j in (1, 3):
                        load_w(j)
                    for cb in range(1, NBLK):
                        load_x(cb)
                else:
                    for cb in range(NBLK):
                        load_x(cb)
                        if cb == 0:
                            for j in range(KS):
                                load_w(j)

                halves = ((0, HB), (HB, R)) if nr == R else ((0, nr),)
                for cb in range(NBLK):
                    ps = psp.tile([128, R, W], f32, name="ps", tag="ps")
                    for jn, j in enumerate(jorder):
                        t = tmpp.tile([128, KS, R, W], bf16, name="t", tag="t")
                        if j % 2 == 0:
                            xt = xpa[cb][:]
                            base = xt.offset + j
                        else:
                            xt = xpb[cb][:]
                            base = xt.offset + (j - 1)
                        xov = bass.AP(
                            xt.tensor, base,
                            [list(xt.ap[0]), [PADH, KS], [PADH, nr], [1, W]],
                        )
                        wsl = wt[:, j * KS]
                        wov = bass.AP(
                            wsl.tensor, wsl.offset,
                            [list(wsl.ap[0]), [R * W, KS], [W, nr], [1, W]],
                        )
                        nc.vector.tensor_mul(t[:, :, :nr], xov, wov)
                        for h0, h1 in halves:
                            for i in range(KS):
                                nc.tensor.matmul(
                                    ps[:, h0:h1],
                                    ident[:],
                                    t[:, i, h0:h1],
                                    start=(jn == 0 and i == 0),
                                    stop=(jn == KS - 1 and i == KS - 1),
                                )
                    # evacuate per PSUM half (lo closes before hi), cast to
                    # bf16; host upcasts the output
                    o = ostp.tile([128, R, W], bf16, name="o", tag="o")
                    q = nc.sync if cb % 2 == 0 else nc.scalar
                    for h0, h1 in halves:
                        nc.scalar.copy(o[:, h0:h1], ps[:, h0:h1])
                        q.dma_start(
                            out[b, cb * 128:(cb + 1) * 128,
                                r0 + h0:r0 + h1, :],
                            o[:, h0:h1],
                        )
    nc.compile()
    return nc


def _build_nc_v5():
    """Channel-block-merged products via per-j shifted x copies (v5).

    The ISA caps engine APs at 3 free dims, which blocks batching the 4
    channel blocks into one DVE instruction as long as x rows live in
    68-wide padded form ((row, col) then needs its own two dims). Fix: the
    host supplies FIVE shifted copies of the padded input, one per
    horizontal tap j, each with contiguous 64-wide rows. (row, col) then
    collapses into one contiguous dim, and one tensor_mul per (chunk, j)
    covers all 4 channel blocks: free dims [cb=4, i=5, rows*cols], the
    weight operand broadcasting over cb with stride 0. 80 product instrs
    instead of 160 -> half the per-instr init/seq overhead and half the
    semaphore traffic on the critical DVE queue. Every copy is 4B-aligned,
    so the odd-j shifted-copy trick disappears too.

    Costs: x HBM traffic rises to 5 64-wide copies with an 8-row chunk
    halo (63MB/core; 124MB total, still well under the DMA budget) and
    chunks shrink to 8 rows (PSUM: 1 bank per channel block, 8 in
    flight).
    """
    import concourse.bass as bass
    import concourse.bacc as bacc
    import concourse.tile as tile
    from concourse import mybir

    f32 = mybir.dt.float32
    bf16 = mybir.dt.bfloat16
    R = 8                # output rows per chunk (PSUM: 512 f32 = 1 bank/cb)
    NCH = H // R         # 8 chunks per sample
    HALO4 = (R + KS - 1) * W   # 768: 12 rows of 64 in a shifted copy
    FRC = R * W          # 512 contiguous (row, col) elems per (cb, i)

    nc = bacc.Bacc("TRN2", target_bir_lowering=False, debug=False, num_devices=NCORES)
    # input: [S, C, j, padded_row, 64] -- five horizontally-shifted copies
    x_in = nc.declare_dram_parameter(
        "input", [S, C, KS, H + 2 * PAD, W], bf16, isOutput=False
    )
    w_in = nc.declare_dram_parameter("weight", [S, CW, KK, H, W], bf16, isOutput=False)
    id_in = nc.declare_dram_parameter("ident", [128, 128], bf16, isOutput=False)
    out = nc.declare_dram_parameter("out", [S, C, H, W], bf16, isOutput=True)

    with tile.TileContext(nc) as tc:
        with (
            tc.tile_pool(name="const", bufs=1) as constp,
            tc.tile_pool(name="xp", bufs=2) as xpp,
            tc.tile_pool(name="wt", bufs=2) as wtp,
            tc.tile_pool(name="tmp", bufs=3) as tmpp,
            tc.tile_pool(name="ts", bufs=2) as tsp,
            tc.tile_pool(name="ost", bufs=4) as ostp,
            tc.tile_pool(name="ps", bufs=8, space="PSUM") as psp,
        ):
            ident = constp.tile([128, 128], bf16)
            nc.scalar.dma_start(ident[:], id_in[:])

            chunks = [(b, k) for b in range(S) for k in range(NCH)]
            for ci, (b, k) in enumerate(chunks):
                first = ci == 0
                last = ci == len(chunks) - 1
                r0 = k * R

                # x5t[p, j, cb, row*col]: one chunk's halo rows of all five
                # shifted copies, rows contiguous per (j, cb)
                x5t = xpp.tile([128, KS, NBLK, HALO4], bf16, tag="x", name="x5t")
                wt = wtp.tile([128, KK, R, W], bf16, name="wt", tag="wt")

                def load_x(cb, j=None):
                    csl = slice(cb * 128, (cb + 1) * 128)
                    q = nc.sync if cb % 2 == 0 else nc.scalar
                    if j is None:
                        q.dma_start(
                            x5t[:, :, cb], x_in[b, csl, :, r0:r0 + R + KS - 1, :]
                        )
                    else:
                        q.dma_start(
                            x5t[:, j, cb], x_in[b, csl, j, r0:r0 + R + KS - 1, :]
                        )

                def load_w(j):
                    wsrc = w_in[b, :, j * KS:(j + 1) * KS, r0:r0 + R, :]
                    jb = slice(j * KS, (j + 1) * KS)
                    nc.sync.dma_start(wt[0:64, jb], wsrc)
                    nc.scalar.dma_start(wt[64:128, jb], wsrc)

                if first:
                    # consumption order: cb0's j0 slice + j0 weights first
                    load_x(0, 0)
                    load_w(0)
                    for j in range(1, KS):
                        load_x(0, j)
                        load_w(j)
                    for cb in range(1, NBLK):
                        load_x(cb)
                else:
                    for cb in range(NBLK):
                        load_x(cb)
                        if cb == 0:
                            for j in range(KS):
                                load_w(j)

                ps = [
                    psp.tile([128, R, W], f32, name="ps", tag="ps")
                    for _ in range(NBLK)
                ]

                def product_merged(j):
                    t = tmpp.tile([128, NBLK, KS, FRC], bf16, name="t", tag="t")
                    xov = bass.AP(
                        x5t.tensor,
                        x5t.offset + j * NBLK * HALO4,
                        [list(x5t.ap[0]), [HALO4, NBLK], [W, KS], [1, FRC]],
                    )
                    wsl = wt[:, j * KS]
                    wov = bass.AP(
                        wsl.tensor, wsl.offset,
                        [list(wsl.ap[0]), [0, NBLK], [FRC, KS], [1, FRC]],
                    )
                    nc.vector.tensor_mul(t[:], xov, wov)
                    return t

                def product_cb(j, cb):
                    t = tsp.tile([128, KS, FRC], bf16, name="tsg", tag="ts")
                    xov = bass.AP(
                        x5t.tensor,
                        x5t.offset + (j * NBLK + cb) * HALO4,
                        [list(x5t.ap[0]), [W, KS], [1, FRC]],
                    )
                    wsl = wt[:, j * KS]
                    wov = bass.AP(
                        wsl.tensor, wsl.offset,
                        [list(wsl.ap[0]), [FRC, KS], [1, FRC]],
                    )
                    nc.vector.tensor_mul(t[:], xov, wov)
                    return t

                def mm(mv, cb, j, i):
                    nc.tensor.matmul(
                        ps[cb][:],
                        ident[:],
                        mv[:, i],
                        start=(j == 0 and i == 0),
                        stop=(j == KS - 1 and i == KS - 1),
                    )

                def store(cb):
                    o = ostp.tile([128, R, W], bf16, name="o", tag="o")
                    nc.scalar.copy(o[:], ps[cb][:])
                    q = nc.sync if cb % 2 == 0 else nc.scalar
                    q.dma_start(
                        out[b, cb * 128:(cb + 1) * 128, r0:r0 + R, :], o[:]
                    )

                for j in range(KS):
                    split = (first and j == 0) or (last and j == KS - 1)
                    if split:
                        for cb in range(NBLK):
                            t = product_cb(j, cb)
                            for i in range(KS):
                                mm(t, cb, j, i)
                            if j == KS - 1:
                                store(cb)
                    else:
                        t = product_merged(j)
                        for cb in range(NBLK):
                            tcb = t[:, cb]
                            for i in range(KS):
                                mm(tcb, cb, j, i)
                            if j == KS - 1:
                                store(cb)
    nc.compile()
    return nc


def _build_nc_v6():
    """Channel-block-merged products from three 66-wide shifted copies.

    Same idea as v5 (collapse (row, col) into one contiguous AP dim so one
    tensor_mul per (chunk, j) covers all 4 channel blocks, weight operand
    broadcast over cb with stride 0), but with the x-traffic blow-up
    fixed: instead of five 64-wide copies, THREE 66-wide copies serve the
    five horizontal taps at even in-row offsets (j0->copy0+0, j1->copy1+0,
    j2->copy2+0, j3->copy1+2, j4->copy2+2), keeping every operand
    4B-aligned for the DVE 2x mode. Rows are 66 wide, so each product
    carries 2 garbage columns per row (3.1% DVE tax) that the PE moving
    APs simply skip; PSUM tiles stay 64-wide and bank-aligned. x HBM
    traffic is 3*66/(2*68) of the old two-copy scheme (~+10MB/core),
    total DMA ~95MB -- comfortably off the critical path, unlike v5.
    40 merged product instrs instead of 160.
    """
    import concourse.bass as bass
    import concourse.bacc as bacc
    import concourse.tile as tile
    from concourse import mybir

    f32 = mybir.dt.float32
    bf16 = mybir.dt.bfloat16
    R = 16               # output rows per chunk
    NCH = H // R         # 4 chunks per sample
    HB = R // 2          # rows per PSUM half
    WC = 66              # copy row width
    FRC = R * WC         # 1056 contiguous (row, col) elems per (cb, i)
    HALOC = (R + KS - 1) * WC  # 1320: 20 rows of 66 per copy
    NCP = 3
    # j -> (copy, even in-row offset)
    JMAP = {0: (0, 0), 1: (1, 0), 2: (2, 0), 3: (1, 2), 4: (2, 2)}

    nc = bacc.Bacc("TRN2", target_bir_lowering=False, debug=False, num_devices=NCORES)
    # input: three 66-wide shifted copies of the zero-padded image
    x_in = nc.declare_dram_parameter(
        "input", [S, C, NCP, H + 2 * PAD, WC], bf16, isOutput=False
    )
    # weight: taps j-major, rows padded to 66 cols (last 2 cols zero)
    w_in = nc.declare_dram_parameter(
        "weight", [S, CW, KK, H, WC], bf16, isOutput=False
    )
    id_in = nc.declare_dram_parameter("ident", [128, 128], bf16, isOutput=False)
    out = nc.declare_dram_parameter("out", [S, C, H, W], bf16, isOutput=True)

    with tile.TileContext(nc) as tc:
        with (
            tc.tile_pool(name="const", bufs=1) as constp,
            tc.tile_pool(name="xp", bufs=2) as xpp,
            tc.tile_pool(name="wt", bufs=1) as wtp,
            tc.tile_pool(name="tmp", bufs=2) as tmpp,
            tc.tile_pool(name="ost", bufs=2) as ostp,
            tc.tile_pool(name="ps", bufs=4, space="PSUM") as psp,
        ):
            ident = constp.tile([128, 128], bf16)
            nc.scalar.dma_start(ident[:], id_in[:])

            chunks = [(b, k) for b in range(S) for k in range(NCH)]
            for ci, (b, k) in enumerate(chunks):
                first = ci == 0
                last = ci == len(chunks) - 1
                r0 = k * R

                # per-copy tiles [p, cb, row*col], +8 elems of slack for the
                # offset-2 products' tail overrun into garbage
                cps = [
                    xpp.tile([128, NBLK * HALOC + 8], bf16, tag=f"cp{c}",
                             name=f"cp{c}")
                    for c in range(NCP)
                ]
                wts = [
                    wtp.tile([128, KS, FRC], bf16, tag=f"wt{j}", name=f"wt{j}")
                    for j in range(KS)
                ]

                def load_cp(cp, cb0=0, ncb=NBLK):
                    q = (nc.sync, nc.scalar, nc.sync)[cp]
                    dst = cps[cp][:, cb0 * HALOC:(cb0 + ncb) * HALOC]
                    dst = bass.AP(
                        dst.tensor, dst.offset,
                        [list(dst.ap[0]), [HALOC, ncb], [1, HALOC]],
                    )
                    src = x_in[b, :, cp, r0:r0 + R + KS - 1, :]
                    sap = bass.AP(
                        src.tensor, src.offset + cb0 * 128 * NCP * PADH * WC,
                        [[NCP * PADH * WC, 128], [128 * NCP * PADH * WC, ncb],
                         [1, HALOC]],
                    )
                    q.dma_start(dst, sap)

                def load_w(j):
                    wsrc = w_in[b, :, j * KS:(j + 1) * KS, r0:r0 + R, :]
                    nc.sync.dma_start(wts[j][0:64], wsrc)
                    nc.scalar.dma_start(wts[j][64:128], wsrc)

                if first:
                    # cb0's copy-0 slice + j0 weights gate the first product
                    load_cp(0, 0, 1)
                    load_w(0)
                    load_cp(0, 1, 3)
                    load_cp(1)
                    load_w(1)
                    load_cp(2)
                    for j in range(2, KS):
                        load_w(j)
                else:
                    for cp in range(NCP):
                        load_cp(cp)
                    for j in range(KS):
                        load_w(j)

                ps = [
                    psp.tile([128, R, W], f32, name="ps", tag="ps")
                    for _ in range(NBLK)
                ]

                for j in range(KS):
                    cp, ofs = JMAP[j]
                    xt = cps[cp]
                    t = tmpp.tile([128, NBLK, KS, FRC], bf16, name="t", tag="t")
                    wsl = wts[j][:]
                    # fill/drain chunks: issue j0/j4 per channel block (into
                    # slices of the same tile) so the first product waits on
                    # 1/4 of the x bytes and the tail drains 10 matmuls, not
                    # 40
                    split = (first and j == 0) or (last and j == KS - 1)
                    if split:
                        for cb in range(NBLK):
                            xov = bass.AP(
                                xt.tensor, xt.offset + cb * HALOC + ofs,
                                [list(xt.ap[0]), [WC, KS], [1, FRC]],
                            )
                            wov = bass.AP(
                                wsl.tensor, wsl.offset,
                                [list(wsl.ap[0]), [FRC, KS], [1, FRC]],
                            )
                            nc.vector.tensor_mul(t[:, cb], xov, wov)
                    else:
                        xov = bass.AP(
                            xt.tensor, xt.offset + ofs,
                            [list(xt.ap[0]), [HALOC, NBLK], [WC, KS], [1, FRC]],
                        )
                        wov = bass.AP(
                            wsl.tensor, wsl.offset,
                            [list(wsl.ap[0]), [0, NBLK], [FRC, KS], [1, FRC]],
                        )
                        nc.vector.tensor_mul(t[:], xov, wov)
                    for cb in range(NBLK):
                        for half in range(2):
                            for i in range(KS):
                                mv = bass.AP(
                                    t.tensor,
                                    t.offset + (cb * KS + i) * FRC
                                    + half * HB * WC,
                                    [list(t.ap[0]), [WC, HB], [1, W]],
                                )
                                nc.tensor.matmul(
                                    ps[cb][:, half * HB:(half + 1) * HB],
                                    ident[:],
                                    mv,
                                    start=(j == 0 and i == 0),
                                    stop=(j == KS - 1 and i == KS - 1),
                                )
                        if j == KS - 1:
                            o = ostp.tile([128, R, W], bf16, name="o", tag="o")
                            nc.scalar.copy(o[:], ps[cb][:])
                            q = nc.sync if cb % 2 == 0 else nc.scalar
                            q.dma_start(
                                out[b, cb * 128:(cb + 1) * 128, r0:r0 + R, :],
                                o[:],
                            )
    nc.compile()
    return nc


def _build_nc_v7():
    """Samples-on-partitions layout (v7).

    Partition p = (sample b = p//64, cw = p%64) instead of a 128-channel
    block. Wins:
    - The 128 weight partition rows are all DISTINCT (two samples' cw
      rows), so the weight partition-duplication double-read disappears:
      w traffic halves to 26MB/core.
    - The share-group dim g (8 groups whose channels reuse the same cw
      weights) moves to the free dims, so ONE tensor_mul per (chunk, tap
      j) covers the whole chunk: free dims [g=8, i=5, row*col] -- exactly
      the ISA's 3-free-dim cap, with the weight operand broadcast over g
      by stride 0 and (row, col) collapsed via host-side per-j shifted
      64-wide x copies (all offsets 0 -> always 4B-aligned, no odd-shift
      copy). 40 product instrs of FD 20480: ~432us, near the 427us DVE
      floor, vs ~453us for the 160-instr per-channel-block structure.
    - Per-j x and w tiles are consumed by that one instr early in the
      chunk, so they single-buffer; their next-chunk reloads self-stagger
      across the chunk (WAR on the DVE product), smoothing DMA demand.

    8-row chunks: per (chunk, g) PSUM tile [128, 8, 64] f32 = exactly one
    bank, 8 groups = all 8 banks; evacuation per g frees its bank long
    before the next chunk's accumulation reaches it. Products stay bf16
    with fp32 PSUM accumulation; output bf16 (host upcasts).
    """
    import concourse.bass as bass
    import concourse.bacc as bacc
    import concourse.tile as tile
    from concourse import mybir

    f32 = mybir.dt.float32
    bf16 = mybir.dt.bfloat16
    G = C // CW          # 8 share groups
    R = 8                # output rows per chunk (1 PSUM bank per group)
    NCH = H // R         # 8 chunks per sample-pair
    HR = R + KS - 1      # 12 halo rows per chunk
    FRC = R * W          # 512 contiguous (row, col) elems
    FH = HR * W          # 768 halo elems per (j, g)

    nc = bacc.Bacc("TRN2", target_bir_lowering=False, debug=False, num_devices=NCORES)
    # input: [p=(b,cw), g, j, padded_row, 64] -- per-j shifted copies
    x_in = nc.declare_dram_parameter(
        "input", [128, G, KS, H + 2 * PAD, W], bf16, isOutput=False
    )
    # weight: [p=(b,cw), j, i, row, col] -- j-major taps, no duplication
    w_in = nc.declare_dram_parameter(
        "weight", [128, KS, KS, H, W], bf16, isOutput=False
    )
    id_in = nc.declare_dram_parameter("ident", [128, 128], bf16, isOutput=False)
    out = nc.declare_dram_parameter("out", [S, C, H, W], bf16, isOutput=True)

    with tile.TileContext(nc) as tc:
        with (
            tc.tile_pool(name="const", bufs=1) as constp,
            tc.tile_pool(name="xp", bufs=1) as xpp,
            tc.tile_pool(name="wt", bufs=1) as wtp,
            tc.tile_pool(name="tmp", bufs=2) as tmpp,
            tc.tile_pool(name="ost", bufs=8) as ostp,
            tc.tile_pool(name="ps", bufs=8, space="PSUM") as psp,
        ):
            ident = constp.tile([128, 128], bf16)
            nc.scalar.dma_start(ident[:], id_in[:])

            # Queue discipline: the sync queue carries ONLY loads, the
            # scalar queue ONLY evacuation copies + store triggers. Both
            # evacuations (waiting on PE's chunk-end stop matmuls) and
            # store triggers resolve late in each chunk; placing any load
            # trigger behind them on the same FIFO queue head-of-line
            # blocks the next chunk's x/w by ~10-15us.
            for ci in range(NCH):
                first = ci == 0
                last = ci == NCH - 1
                r0 = ci * R

                HG = G // 2   # groups per product half
                xts = [
                    xpp.tile([128, G, FH], bf16, tag=f"x{j}", name=f"x{j}")
                    for j in range(KS)
                ]
                wts = [
                    wtp.tile([128, KS, FRC], bf16, tag=f"w{j}", name=f"w{j}")
                    for j in range(KS)
                ]

                def load_x(j, h):
                    # per-half loads so each half-product waits only on its
                    # own DMA
                    g0 = h * HG
                    nc.sync.dma_start(
                        xts[j][:, g0:g0 + HG],
                        x_in[:, g0:g0 + HG, j, r0:r0 + HR, :],
                    )

                def load_w(j):
                    nc.sync.dma_start(wts[j][:], w_in[:, j, :, r0:r0 + R, :])

                if first:
                    # phase-A consumption order: h0's x slices + weights
                    # first, h1's x afterwards
                    load_x(0, 0)
                    load_w(0)
                    for j in range(1, KS):
                        load_x(j, 0)
                        load_w(j)
                    for j in range(KS):
                        load_x(j, 1)
                else:
                    for j in range(KS):
                        load_x(j, 0)
                        load_x(j, 1)
                        load_w(j)

                ps = [
                    psp.tile([128, R, W], f32, name="ps", tag="ps")
                    for _ in range(G)
                ]

                def product(j, h):
                    # half-group product: staggers PE consumption, PSUM
                    # completion and evacuation across the chunk instead of
                    # bunching them at the chunk boundary
                    t = tmpp.tile([128, HG, KS, FRC], bf16, name="t",
                                  tag=f"t{h}")
                    xt = xts[j]
                    xov = bass.AP(
                        xt.tensor, xt.offset + h * HG * FH,
                        [list(xt.ap[0]), [FH, HG], [W, KS], [1, FRC]],
                    )
                    wsl = wts[j][:]
                    wov = bass.AP(
                        wsl.tensor, wsl.offset,
                        [list(wsl.ap[0]), [0, HG], [FRC, KS], [1, FRC]],
                    )
                    nc.vector.tensor_mul(t[:], xov, wov)
                    return t

                def store(g):
                    o = ostp.tile([128, R, W], bf16, name="o", tag="o")
                    nc.scalar.copy(o[:], ps[g][:])
                    # p = (b, cw) -> out[b, g*64+cw, r0:r0+R, :]
                    osl = out[0, g * CW, r0]
                    nc.scalar.dma_start(
                        bass.AP(
                            osl.tensor, osl.offset,
                            [[C * H * W, S], [H * W, CW], [1, FRC]],
                        ),
                        o[:],
                    )

                # phase-split: all 5 taps for groups 0-3, then groups 4-7.
                # Phase A's four PSUM banks complete and evacuate MID-chunk
                # (overlapped with phase B's compute), so the next chunk's
                # accumulation never waits on this chunk's evacuation and
                # the PE/DVE pipeline crosses chunk boundaries without a
                # bubble.
                for h in range(2):
                    for j in range(KS):
                        t = product(j, h)
                        for g4 in range(HG):
                            g = h * HG + g4
                            for i in range(KS):
                                nc.tensor.matmul(
                                    ps[g][:],
                                    ident[:],
                                    t[:, g4, i],
                                    start=(j == 0 and i == 0),
                                    stop=(j == KS - 1 and i == KS - 1),
                                )
                            if j == KS - 1:
                                store(g)
    nc.compile()
    return nc


def _variant():
    return os.environ.get("BASS_KERNEL_VARIANT", "v3")


def _get_nc():
    v = _variant()
    if v not in _STATE:
        if v == "v7":
            _STATE[v] = _build_nc_v7()
        elif v == "v6":
            _STATE[v] = _build_nc_v6()
        elif v == "v5":
            _STATE[v] = _build_nc_v5()
        elif v == "v3":
            _STATE[v] = _build_nc_v3()
        elif v == "bf16":
            _STATE[v] = _build_nc_bf16()
        else:
            _STATE[v] = _build_nc()
    return _STATE[v]


def run(input, weight, trace=False):
    """Run on 8 NeuronCores; returns (output, BassKernelResults)."""
    from concourse.bass_utils import run_bass_kernel_spmd

    assert input.shape == (B, C, H, W), input.shape
    assert weight.shape == (B, CW, KK, H * W), weight.shape
    if _variant() == "v7":
        import ml_dtypes

        dt = ml_dtypes.bfloat16
        # Partition = (sample, cw): per core, [128, g, j, row, 64] with
        # per-j shifted 64-wide copies of the zero-padded image.
        G = C // CW
        xpad = np.zeros((B, C, PADH, PADH), dtype=dt)
        xpad[:, :, PAD:PAD + H, PAD:PAD + W] = np.asarray(
            input, dtype=np.float32
        ).astype(dt)
        xv = xpad.reshape(NCORES, S, G, CW, PADH, PADH)
        xv = np.ascontiguousarray(xv.transpose(0, 1, 3, 2, 4, 5))
        xv = xv.reshape(NCORES, 128, G, PADH, PADH)
        inp = np.empty((NCORES, 128, G, 5, PADH, W), dtype=dt)
        for j in range(5):
            inp[:, :, :, j] = xv[..., j:j + W]
    elif _variant() == "v6":
        import ml_dtypes

        dt = ml_dtypes.bfloat16
        # Three 66-wide shifted copies of the zero-padded input with
        # contiguous rows: the five horizontal taps read them at even
        # in-row offsets, keeping the DVE 2x-mode alignment.
        xpad = np.zeros((B, C, PADH, PADH), dtype=dt)
        xpad[:, :, PAD:PAD + H, PAD:PAD + W] = np.asarray(
            input, dtype=np.float32
        ).astype(dt)
        inp = np.empty((B, C, 3, PADH, 66), dtype=dt)
        for c in range(3):
            inp[:, :, c] = xpad[:, :, :, c:c + 66]
    elif _variant() == "v5":
        import ml_dtypes

        dt = ml_dtypes.bfloat16
        # Five horizontally-shifted copies of the zero-padded input, each
        # with contiguous 64-wide rows: x5[b, c, j, r, :] = xpad[b, c, r,
        # j:j+64]. Lets the kernel collapse (row, col) into one AP dim.
        xpad = np.zeros((B, C, PADH, PADH), dtype=dt)
        xpad[:, :, PAD:PAD + H, PAD:PAD + W] = np.asarray(
            input, dtype=np.float32
        ).astype(dt)
        inp = np.empty((B, C, 5, PADH, W), dtype=dt)
        for j in range(5):
            inp[:, :, j] = xpad[:, :, :, j:j + W]
    elif _variant() in ("bf16", "v3"):
        import ml_dtypes

        dt = ml_dtypes.bfloat16
        # Pre-pad on the host: [B, C, 68*68+1] with zero borders, so the
        # kernel's two shifted SBUF copies are fully contiguous DMAs.
        inp = np.zeros((B, C, PADH * PADH + 1), dtype=dt)
        view = inp[:, :, :PADH * PADH].reshape(B, C, PADH, PADH)
        view[:, :, PAD:PAD + H, PAD:PAD + W] = np.asarray(
            input, dtype=np.float32
        ).astype(dt)
    else:
        dt = np.float32
        inp = np.ascontiguousarray(np.asarray(input, dtype=np.float32))
    wgt = np.asarray(weight, dtype=np.float32).astype(dt).reshape(B, CW, KK, H, W)
    if _variant() in ("bf16", "v3", "v5", "v6", "v7"):
        # j-major tap order: slot j*5+i holds original tap i*5+j, so each
        # horizontal shift's 5 vertical taps are contiguous in SBUF
        wgt = wgt.reshape(B, CW, KS, KS, H, W).transpose(0, 1, 3, 2, 4, 5)
        wgt = wgt.reshape(B, CW, KK, H, W)
    if _variant() == "v6":
        # rows padded to 66 cols (zeros) to match the 66-wide x copies
        w66 = np.zeros((B, CW, KK, H, 66), dtype=dt)
        w66[..., :W] = wgt
        wgt = w66
    if _variant() == "v7":
        # [core, p=(b,cw), j, i, row, col]
        wgt = wgt.reshape(NCORES, 128, KS, KS, H, W)
    wgt = np.ascontiguousarray(wgt)
    ident = np.eye(128, dtype=dt)

    nc = _get_nc()
    core_ids = list(range(NCORES))
    if _variant() == "v7":
        in_maps = [
            {"input": inp[c], "weight": wgt[c], "ident": ident}
            for c in core_ids
        ]
    else:
        in_maps = [
            {
                "input": inp[c * S:(c + 1) * S],
                "weight": wgt[c * S:(c + 1) * S],
                "ident": ident,
            }
            for c in core_ids
        ]
    res = run_bass_kernel_spmd(nc, in_maps, core_ids, trace=trace)
    outp = np.concatenate([res.results[c]["out"] for c in core_ids], axis=0)
    outp = np.ascontiguousarray(outp.astype(np.float32))
    return outp, res


def kernel(input, weight):
    trace = bool(int(os.environ.get("BASS_KERNEL_TRACE", "0")))
    outp, _ = run(input, weight, trace=trace)
    return outp

